# revision 1
# baseline (speedup 1.0000x reference)
"""Mistral-style MHA prefill kernel for Trainium2, 8-way tensor-parallel over heads.

Problem (hardcoded): B=1, S=2048, DIM=4096, 32 q-heads / 8 kv-heads, head_dim=128,
sliding window 2048 (== S, so the mask is exactly causal), rope theta 1e4.

Sharding: core c owns q-heads [4c, 4c+4) and kv-head c. wq/wk/wv are sharded on the
head axis, wo on its input (head) axis; each core computes a full-shape partial
output and the host sums the 8 partials (row-parallel linear + host all-reduce).

Layout strategy (all chosen host-side so the device never transposes activations):
  - x is passed pre-transposed xT [DIM, S]; projections run as W @ x -> [feat, S],
    so Q^T/K^T/V^T [128, S] per head come straight out of PSUM.
  - head_dim is permuted per 32-partition quadrant (16 re rows, then 16 im rows)
    so RoPE pairs sit +-16 apart inside a quadrant: the rotation is a
    stream_shuffle + two muls + one fused sign-multiply-add, all full-width.
    The permutation is score-invariant (applied consistently to Q and K).
  - 1/sqrt(head_dim) is folded into the rope cos/sin tables as sqrt(scale).
  - scores are computed transposed, S_T[k, q] (k on partitions), per 512-wide
    q-block: softmax exp runs on ScalarE, the denominator accumulates on the PE
    via a ones-column matmul into a single PSUM row, and P^T never needs a
    transpose: out^T[dv, q] accrues in PSUM with V (normal layout, 16 PE
    transposes) as the stationary operand.
  - causality at 128-col granularity: (k, q-block) pairs with k > q are never
    computed; diagonal blocks are masked with suffix slices of one [128, 512]
    zeros|triangle tile. All matmuls are exactly N=512 — float32r only reaches
    full rate (1 cyc/row) at 512 on hardware.
  - normalization: recip(denom) is broadcast across partitions with a K=1 outer
    product matmul and fused into the PSUM->SBUF eviction of out^T.
  - wo projection: out^T slices are the stationary operand, woT [512, 4096] the
    moving one; PSUM [s_tile, dout] evicted to DRAM in 1024-wide stripes.

All matmuls run in float32r (reduced-precision fp32, ~1.5e-4 relative error per
matmul on HW; end-to-end ~3e-4 against the fp32 reference).
"""

import numpy as np

B = 1
S = 2048
DIM = 4096
N_HEADS = 32
N_KV = 8
DH = 128
NCORES = 8
HPC = N_HEADS // NCORES  # q heads per core
FQKV = HPC * DH + 2 * DH  # 768 projection rows per core
NKT = S // DH  # 16 k tiles
NQB = S // 512  # 4 q blocks
NDCH = DIM // DH  # 32 contraction chunks

_PROGRAM = None

# stream_shuffle mask: swap 16-partition halves within each 32-partition quadrant
_SWAP16 = [(i + 16) % 32 for i in range(32)]


def _head_perm():
    """Permutation of head_dim rows: quadrant q holds [re_16q..re_16q+15,
    im_16q..im_16q+15], so RoPE pairs are +-16 apart within a quadrant."""
    p = np.empty(DH, dtype=np.int64)
    for row in range(DH):
        q, j = divmod(row, 32)
        i = 16 * q + (j % 16)  # rope pair index
        p[row] = 2 * i + (0 if j < 16 else 1)
    return p


def _build_program():
    import concourse.bacc as bacc
    import concourse.mybir as mybir
    import concourse.tile as tile

    F32 = mybir.dt.float32
    F32R = mybir.dt.float32r
    EXP = mybir.ActivationFunctionType.Exp

    nc = bacc.Bacc("TRN2", target_bir_lowering=False, debug=False,
                   enable_asserts=False)

    xT = nc.dram_tensor("xT", [DIM, S], F32R, kind="ExternalInput")
    wqkvT = nc.dram_tensor("wqkvT", [DIM, FQKV], F32R, kind="ExternalInput")
    woT = nc.dram_tensor("woT", [HPC * DH, DIM], F32R, kind="ExternalInput")
    csA_d = nc.dram_tensor("csA", [DH, S], F32R, kind="ExternalInput")
    csB_d = nc.dram_tensor("csB", [DH, S], F32R, kind="ExternalInput")
    sign_d = nc.dram_tensor("sign", [DH, 1], F32R, kind="ExternalInput")
    tri512_d = nc.dram_tensor("tri512", [DH, 512], F32R, kind="ExternalInput")
    ident_d = nc.dram_tensor("ident", [DH, DH], F32R, kind="ExternalInput")
    ones128_d = nc.dram_tensor("ones128", [DH, DH], F32R, kind="ExternalInput")
    out_d = nc.dram_tensor("out", [S, DIM], F32, kind="ExternalOutput")

    with tile.TileContext(nc) as tc:
        with (
            tc.tile_pool(name="consts", bufs=1) as cpool,
            tc.tile_pool(name="persist", bufs=1) as ppool,
        ):
            csA_sb = cpool.tile([DH, S], F32R)
            csB_sb = cpool.tile([DH, S], F32R)
            sign_sb = cpool.tile([DH, 1], F32R)
            tri512_sb = cpool.tile([DH, 512], F32R)
            ident_sb = cpool.tile([DH, DH], F32R)
            ones128_sb = cpool.tile([DH, DH], F32R)

            qt = [ppool.tile([DH, S], F32R, name=f"qt{h}") for h in range(HPC)]
            kt = ppool.tile([DH, S], F32R)
            vn = ppool.tile([DH, S], F32R)  # V in normal layout, 16 [128,128] chunks

            # ------- Fused QKV + attention: after s-block i completes, the
            # attention q-block i (all heads) runs while the next s-block's
            # activations stream in. One shared PSUM pool (tag "ps") serves the
            # six QKV accumulators and the attention score/out/denominator
            # tiles. otn aliases qt: attention block b is the last reader of
            # qt[h][:, b*512:(b+1)*512], so the normalized out^T overwrites it.
            otn = qt
            with (
                tc.tile_pool(name="xin", bufs=8) as xpool,
                tc.tile_pool(name="mps", bufs=7, space="PSUM") as mps,
                tc.tile_pool(name="trps", bufs=1, space="PSUM") as trps,
                tc.tile_pool(name="ropet", bufs=2) as rtp,
                tc.tile_pool(name="rawsb", bufs=5) as rawpool,
                tc.tile_pool(name="vtt", bufs=1) as vtp,
                tc.tile_pool(name="esb", bufs=3) as epool,
                tc.tile_pool(name="bcsb", bufs=1) as bcpool,
            ):
                def emit_sblock(sb_i, w_sb):
                    col = slice(sb_i * 512, (sb_i + 1) * 512)
                    ps = [mps.tile([DH, 512], F32, name=f"ps{f}", tag="ps")
                          for f in range(6)]
                    for d in range(NDCH):
                        if sb_i == 0:
                            # weight loads ride the ACT HWDGE ring, concurrent
                            # with the x loads on the SP ring
                            nc.scalar.dma_start(
                                w_sb[:, d * FQKV:(d + 1) * FQKV],
                                wqkvT[d * DH:(d + 1) * DH, :])
                        xt = xpool.tile([DH, 512], F32R, name="xt", tag="xt")
                        # split the x stream across HWDGE (sync) and SWDGE
                        # (gpsimd) so it sustains > one ring's bandwidth
                        xeng = nc.sync if d % 2 == 0 else nc.gpsimd
                        xeng.dma_start(xt[:], xT[d * DH:(d + 1) * DH, col])
                        if sb_i == 0 and d == 8:
                            # constants are first needed by the sb0 evictions;
                            # don't let them delay the first matmul
                            nc.sync.dma_start(csA_sb[:], csA_d[:])
                            nc.sync.dma_start(csB_sb[:], csB_d[:])
                            nc.sync.dma_start(sign_sb[:], sign_d[:])
                        if sb_i == 0 and d == 16:
                            nc.sync.dma_start(tri512_sb[:], tri512_d[:])
                            nc.sync.dma_start(ident_sb[:], ident_d[:])
                            nc.sync.dma_start(ones128_sb[:], ones128_d[:])
                        for f in range(6):
                            nc.tensor.matmul(
                                ps[f][:],
                                w_sb[:, d * FQKV + f * DH: d * FQKV + (f + 1) * DH],
                                xt[:], start=(d == 0), stop=(d == NDCH - 1))
                    # Fast raw PSUM->SBUF evictions (alternating ACT/DVE) free
                    # the accumulator banks quickly; RoPE runs later from SBUF.
                    # V first on ACT so the PE transposes right after the
                    # last accumulation matmul; Q/K raw evictions split ACT/DVE
                    vt_t = vtp.tile([DH, 512], F32R, name="vt_t", tag="vt")
                    nc.scalar.copy(vt_t[:], ps[5][:])
                    raws = {}
                    for i, f in enumerate([0, 4, 2, 1, 3]):
                        raw = rawpool.tile([DH, 512], F32R, name="raw", tag="raw")
                        raws[f] = raw
                        if i % 2 == 1:
                            nc.scalar.copy(raw[:], ps[f][:])
                        else:
                            nc.vector.tensor_copy(raw[:], ps[f][:])
                    for t in range(4):
                        tp = trps.tile([DH, DH], F32R, name="tp", tag="tp")
                        nc.tensor.transpose(tp[:], vt_t[:, t * DH:(t + 1) * DH],
                                            ident_sb[:])
                        j = sb_i * 4 + t
                        nc.vector.tensor_copy(vn[:, j * DH:(j + 1) * DH], tp[:])
                    return raws

                def emit_rope(f, sb_i, raw):
                    # head_dim permuted so pairs sit +-16 apart within each
                    # 32-partition quadrant: dest = p1 + sign*p3 where
                    # p1 = q*cos, p3 = halfswap(q)*sin.
                    col = slice(sb_i * 512, (sb_i + 1) * 512)
                    dest = qt[f] if f < HPC else kt
                    qs_t = rtp.tile([DH, 512], F32, name="qs_t", tag="qs")
                    p1 = rtp.tile([DH, 512], F32, name="p1", tag="p1")
                    nc.vector.stream_shuffle(qs_t[:], raw[:], _SWAP16)
                    nc.vector.tensor_mul(p1[:], raw[:], csA_sb[:, col])
                    nc.vector.tensor_mul(qs_t[:], qs_t[:], csB_sb[:, col])
                    nc.vector.scalar_tensor_tensor(
                        dest[:, col], qs_t[:], sign_sb[:], p1[:],
                        mybir.AluOpType.mult, mybir.AluOpType.add)

                def emit_block(h, b):
                    cb = slice(b * 512, (b + 1) * 512)
                    nk = 4 * b + 4  # k tiles contributing to this q block
                    ot_b = mps.tile([DH, 512], F32, name="ot", tag="ps")
                    dn_b = mps.tile([DH, 512], F32, name="dn", tag="ps")
                    e_tiles = [None] * nk

                    def emit_scores(k):
                        e = epool.tile([DH, 512], F32R, name="E", tag="E")
                        e_tiles[k] = e
                        sp = mps.tile([DH, 512], F32, name="sp", tag="ps")
                        nc.tensor.matmul(
                            sp[:], kt[:, k * DH:(k + 1) * DH],
                            qt[h][:, cb], start=True, stop=True)
                        nc.scalar.activation(e[:], sp[:], EXP)
                        if k // 4 == b:
                            # diagonal block: zero the disallowed prefix
                            w = (k % 4 + 1) * DH
                            nc.vector.tensor_mul(
                                e[:, :w], e[:, :w], tri512_sb[:, 512 - w:])

                    def emit_pv(k):
                        e = e_tiles[k]
                        st, sp_ = (k == 0), (k == nk - 1)
                        nc.tensor.matmul(ot_b[:], vn[:, k * DH:(k + 1) * DH],
                                         e[:], start=st, stop=sp_)
                        nc.tensor.matmul(dn_b[:], ones128_sb[:], e[:],
                                         start=st, stop=sp_)

                    # 2-deep software pipeline: scores run two steps ahead of
                    # PV/DN so exp/mask latency never stalls the PE
                    emit_scores(0)
                    if nk > 1:
                        emit_scores(1)
                    for k in range(2, nk):
                        emit_scores(k)
                        emit_pv(k - 2)
                    if nk > 1:
                        emit_pv(nk - 2)
                    emit_pv(nk - 1)

                    # normalization tail (DVE only, off the PE critical path);
                    # overwrites the dead qt[h] block in place
                    bc_sb = bcpool.tile([DH, 512], F32R, name="bc_sb", tag="bcs")
                    with nc.allow_low_precision(reason="f32r == f32 bits"):
                        nc.vector.reciprocal(bc_sb[:], dn_b[:])
                    nc.vector.tensor_mul(otn[h][:, cb], ot_b[:], bc_sb[:])

                with tc.tile_pool(name="wsb", bufs=1) as wpool:
                    w_sb = wpool.tile([DH, NDCH * FQKV], F32R)
                    for sb_i in range(NQB):
                        raws = emit_sblock(sb_i, w_sb)
                        if sb_i < NQB - 1:
                            for f in [0, 4, 1, 2, 3]:
                                emit_rope(f, sb_i, raws[f])

                # wo weights take over the region the qkv weights vacated; the
                # 8MB ride the sync ring during attention
                with (
                    tc.tile_pool(name="wosb", bufs=1) as wopool,
                    tc.tile_pool(name="evsb", bufs=3) as evpool,
                ):
                    wo_sb = wopool.tile([DH, HPC * DIM], F32R)
                    for ch in range(HPC):
                        nc.sync.dma_start(wo_sb[:, ch * DIM:(ch + 1) * DIM],
                                          woT[ch * DH:(ch + 1) * DH, :])

                    # attention: the last s-block's RoPE (only consumed by the
                    # b=3 blocks) interleaves between heads so its DVE chain
                    # never gates the first scores
                    emit_rope(0, 3, raws[0])
                    emit_rope(4, 3, raws[4])
                    for h in range(HPC):
                        if h >= 1:
                            emit_rope(h, 3, raws[h])
                        for b in range(NQB):
                            emit_block(h, b)

                    # ---------------- Output projection ----------------------
                    for st in range(NKT):
                        scol = slice(st * DH, (st + 1) * DH)
                        for dh_i in range(2):
                            pw = [mps.tile([DH, 512], F32, name=f"pw{j}",
                                           tag="ps") for j in range(4)]
                            for h in range(HPC):
                                for j in range(4):
                                    nc.tensor.matmul(
                                        pw[j][:],
                                        otn[h][:, scol],
                                        wo_sb[:, h * DIM + dh_i * 2048 + j * 512:
                                              h * DIM + dh_i * 2048 + (j + 1) * 512],
                                        start=(h == 0), stop=(h == HPC - 1))
                            for j2 in range(2):
                                # evict PSUM via ACT/DVE alternately (DMA cannot
                                # read PSUM), write out in 1024-wide stripes on
                                # alternating HWDGE rings
                                ev = evpool.tile([DH, 1024], F32, name="ev",
                                                 tag="ev")
                                if j2 == 0:
                                    nc.scalar.copy(ev[:, 0:512], pw[2 * j2][:])
                                    nc.vector.tensor_copy(ev[:, 512:1024],
                                                          pw[2 * j2 + 1][:])
                                else:
                                    nc.vector.tensor_copy(ev[:, 0:512],
                                                          pw[2 * j2][:])
                                    nc.scalar.copy(ev[:, 512:1024],
                                                   pw[2 * j2 + 1][:])
                                dst = out_d[scol, dh_i * 2048 + j2 * 1024:
                                            dh_i * 2048 + (j2 + 1) * 1024]
                                if (st + j2) % 2 == 0:
                                    nc.sync.dma_start(dst, ev[:])
                                else:
                                    nc.scalar.dma_start(dst, ev[:])

    nc.compile()
    return nc


def get_program():
    global _PROGRAM
    if _PROGRAM is None:
        _PROGRAM = _build_program()
    return _PROGRAM


def make_in_maps(inputs):
    """Host-side sharding / layout prep. Returns one input dict per core."""
    x = np.asarray(inputs["x"], dtype=np.float32)
    wq = np.asarray(inputs["wq"], dtype=np.float32)
    wk = np.asarray(inputs["wk"], dtype=np.float32)
    wv = np.asarray(inputs["wv"], dtype=np.float32)
    wo = np.asarray(inputs["wo"], dtype=np.float32)
    cos = np.asarray(inputs["freqs_cos"], dtype=np.float32)  # (S, 64)
    sin = np.asarray(inputs["freqs_sin"], dtype=np.float32)

    xT = np.ascontiguousarray(x.reshape(S, DIM).T)  # (DIM, S)

    perm = _head_perm()
    sq = np.float32(DH ** -0.25)  # sqrt of 1/sqrt(head_dim), folded into Q and K
    rows = np.arange(DH)
    pair_idx = 16 * (rows // 32) + (rows % 32) % 16
    csA = np.ascontiguousarray(cos.T[pair_idx] * sq)          # (128, S)
    csB = np.ascontiguousarray(sin.T[pair_idx] * sq)
    sign = np.where((rows % 32) < 16, -1.0, 1.0).astype(np.float32).reshape(DH, 1)
    tri = np.triu(np.ones((DH, DH), dtype=np.float32))
    tri512 = np.concatenate([np.zeros((DH, 512 - DH), np.float32), tri], axis=1)
    ident = np.eye(DH, dtype=np.float32)
    ones128 = np.ones((DH, DH), dtype=np.float32)

    wqh = wq.reshape(N_HEADS, DH, DIM)[:, perm, :]
    wkh = wk.reshape(N_KV, DH, DIM)[:, perm, :]
    wvh = wv.reshape(N_KV, DH, DIM)

    in_maps = []
    for c in range(NCORES):
        w_c = np.concatenate(
            [wqh[HPC * c:HPC * (c + 1)].reshape(HPC * DH, DIM),
             wkh[c], wvh[c]], 0)  # (768, DIM)
        wqkvT = np.ascontiguousarray(w_c.T)  # (DIM, 768)
        woT = np.ascontiguousarray(wo[:, HPC * DH * c:HPC * DH * (c + 1)].T)
        in_maps.append({
            "xT": xT, "wqkvT": wqkvT, "woT": woT,
            "csA": csA, "csB": csB, "sign": sign, "tri512": tri512,
            "ident": ident, "ones128": ones128,
        })
    return in_maps


def _ensure_ntff_hook():
    """The agent image's `antenv` lacks `axon_hooks`; recreate it so
    run_bass_kernel_spmd(trace=True) can capture NTFF profiles. Mirrors
    trn_agent_boot/trn_boot.py::_ntff_profile_via_ctypes."""
    import sys
    try:
        from antenv.axon_hooks import get_axon_ntff_profile_hook  # noqa: F401
        return
    except ImportError:
        pass
    import contextlib
    import ctypes
    import types

    so_path = "/opt/axon/libaxon_pjrt.so"
    hook = None
    try:
        lib = ctypes.CDLL(so_path)
        if hasattr(lib, "axon_start_nrt_profile"):
            lib.axon_start_nrt_profile.argtypes = [
                ctypes.POINTER(ctypes.c_int64), ctypes.c_size_t]
            lib.axon_start_nrt_profile.restype = ctypes.c_int64
            lib.axon_stop_nrt_profile.argtypes = [ctypes.c_char_p]
            lib.axon_stop_nrt_profile.restype = ctypes.c_int64

            @contextlib.contextmanager
            def _hook(output_dir, device_ids):
                import jax
                jax.devices()
                if device_ids:
                    ids = (ctypes.c_int64 * len(device_ids))(*device_ids)
                    rc = lib.axon_start_nrt_profile(ids, len(device_ids))
                else:
                    rc = lib.axon_start_nrt_profile(None, 0)
                if rc != 0:
                    raise RuntimeError(f"axon_start_nrt_profile rc={rc}")
                try:
                    yield
                finally:
                    n = lib.axon_stop_nrt_profile(str(output_dir).encode())
                    print(f"profile: {n} file(s) written to {output_dir}")

            hook = _hook
    except OSError:
        pass

    mod = types.ModuleType("antenv.axon_hooks")
    mod._hook = hook
    mod.get_axon_ntff_profile_hook = lambda: mod._hook
    mod.set_axon_ntff_profile_hook = lambda h: setattr(mod, "_hook", h)
    sys.modules["antenv.axon_hooks"] = mod


def run(inputs, trace=False):
    from concourse.bass_utils import run_bass_kernel_spmd
    if trace:
        _ensure_ntff_hook()
    nc = get_program()
    in_maps = make_in_maps(inputs)
    res = run_bass_kernel_spmd(nc, in_maps, core_ids=list(range(NCORES)),
                               trace=trace)
    acc = np.zeros((S, DIM), dtype=np.float32)
    for r in res.results:
        acc += np.asarray(r["out"], dtype=np.float32)
    return acc.reshape(B, S, DIM), res


def kernel(**inputs):
    out, _ = run(inputs, trace=False)
    return out



# revision 8
# speedup vs baseline: 1.2290x; 1.2290x over previous
"""Mistral-style MHA prefill kernel for Trainium2, 8-way tensor-parallel over heads.

Problem (hardcoded): B=1, S=2048, DIM=4096, 32 q-heads / 8 kv-heads, head_dim=128,
sliding window 2048 (== S, so the mask is exactly causal), rope theta 1e4.

Sharding: core c owns q-heads [4c, 4c+4) and kv-head c. wq/wk/wv are sharded on the
head axis, wo on its input (head) axis; each core computes a full-shape partial
output and the host sums the 8 partials (row-parallel linear + host all-reduce).

v2 design (vs the fp32r baseline at ~520 us):
  - All matmul operands are bf16 (PSUM accumulation stays fp32). Same PE rate as
    float32r (1 cyc/row) but halves DMA traffic, so s-block 0 is no longer
    DMA-bound, and unlocks the 2x/4x DVE perf modes for 2-byte dtypes.
    Numpy-simulated end-to-end rel err of the full-bf16 scheme: 4.4e-3.
  - The 160 softmax-denominator matmuls are off the PE: e-tiles accumulate
    elementwise into esum on the DVE (bf16, 2x mode), and ONE ones-matmul per
    (head, block) reduces esum across partitions (broadcast for free).
  - reciprocal() [3.4 us!] -> reciprocal_approx_fast() [~0.7 us, 18 bits].
  - With the denominator matmuls gone, attention is ACT-bound (exp = 674 ns/tile
    vs 426 ns of PE work). So the output projection of q-block b-1 is software-
    pipelined INTO the attention stream of q-block b: its matmuls (no exp
    dependency) fill the PE while ACT catches up on exps. Block order is b-outer,
    head-inner; out-proj of block 3 forms a PE-dense tail.
  - Layouts as baseline: x pre-transposed, per-quadrant rope permutation with
    stream_shuffle +-16, sqrt(scale) folded into the rope tables, transposed
    scores S_T[k, q], causality at (k-tile, 512-q-block) granularity, diagonal
    masked with a zeros|triangle tile.
"""

import numpy as np

B = 1
S = 2048
DIM = 4096
N_HEADS = 32
N_KV = 8
DH = 128
NCORES = 8
HPC = N_HEADS // NCORES  # q heads per core
FQKV = HPC * DH + 2 * DH  # 768 projection rows per core
NKT = S // DH  # 16 k tiles
NQB = S // 512  # 4 q blocks
NDCH = DIM // DH  # 32 contraction chunks

_PROGRAM = None

# stream_shuffle mask: swap 16-partition halves within each 32-partition quadrant
_SWAP16 = [(i + 16) % 32 for i in range(32)]


def _head_perm():
    """Permutation of head_dim rows: quadrant q holds [re_16q..re_16q+15,
    im_16q..im_16q+15], so RoPE pairs are +-16 apart within a quadrant."""
    p = np.empty(DH, dtype=np.int64)
    for row in range(DH):
        q, j = divmod(row, 32)
        i = 16 * q + (j % 16)  # rope pair index
        p[row] = 2 * i + (0 if j < 16 else 1)
    return p


def _build_program():
    import concourse.bacc as bacc
    import concourse.mybir as mybir
    import concourse.tile as tile

    F32 = mybir.dt.float32
    BF16 = mybir.dt.bfloat16
    EXP = mybir.ActivationFunctionType.Exp

    nc = bacc.Bacc("TRN2", target_bir_lowering=False, debug=False,
                   enable_asserts=False)

    xT = nc.dram_tensor("xT", [DIM, S], BF16, kind="ExternalInput")
    wqkvT = nc.dram_tensor("wqkvT", [DIM, FQKV], BF16, kind="ExternalInput")
    woT = nc.dram_tensor("woT", [HPC * DH, DIM], BF16, kind="ExternalInput")
    csA_d = nc.dram_tensor("csA", [DH, S], BF16, kind="ExternalInput")
    csB_d = nc.dram_tensor("csB", [DH, S], BF16, kind="ExternalInput")
    sign_d = nc.dram_tensor("sign", [DH, 1], F32, kind="ExternalInput")
    tri512_d = nc.dram_tensor("tri512", [DH, 512], BF16, kind="ExternalInput")
    ident_d = nc.dram_tensor("ident", [DH, DH], BF16, kind="ExternalInput")
    ones128_d = nc.dram_tensor("ones128", [DH, DH], BF16, kind="ExternalInput")
    out_d = nc.dram_tensor("out", [S, DIM], F32, kind="ExternalOutput")

    with tile.TileContext(nc) as tc:
        with (
            tc.tile_pool(name="consts", bufs=1) as cpool,
            tc.tile_pool(name="persist", bufs=1) as ppool,
            tc.tile_pool(name="xin", bufs=8) as xpool,
            tc.tile_pool(name="ropet", bufs=2) as rtp,
            tc.tile_pool(name="rawsb", bufs=5) as rawpool,
            tc.tile_pool(name="vtt", bufs=1) as vtp,
        ):
            csA_sb = cpool.tile([DH, S], BF16)
            csB_sb = cpool.tile([DH, S], BF16)
            sign_sb = cpool.tile([DH, 1], F32)
            tri512_sb = cpool.tile([DH, 512], BF16)
            ident_sb = cpool.tile([DH, DH], BF16)
            ones128_sb = cpool.tile([DH, DH], BF16)

            qt = [ppool.tile([DH, S], BF16, name=f"qt{h}") for h in range(HPC)]
            kt = ppool.tile([DH, S], BF16)
            vn = ppool.tile([DH, S], BF16)  # V normal layout, 16 [128,128] chunks
            wo_sb = ppool.tile([DH, HPC * DIM], BF16)
            # otn aliases qt: attention block b is the last reader of
            # qt[h][:, b*512:(b+1)*512], so the normalized out^T overwrites it.
            otn = qt

            def emit_rope(f, sb_i, raw):
                # head_dim permuted so pairs sit +-16 apart within each
                # 32-partition quadrant: dest = p1 + sign*p3 where
                # p1 = q*cos, p3 = halfswap(q)*sin. All bf16 SBUF operands so
                # the muls run 2x and the stt 4x on the DVE.
                col = slice(sb_i * 512, (sb_i + 1) * 512)
                dest = qt[f] if f < HPC else kt
                qs_t = rtp.tile([DH, 512], BF16, name="qs_t", tag="qs")
                p1 = rtp.tile([DH, 512], BF16, name="p1", tag="p1")
                nc.vector.stream_shuffle(qs_t[:], raw[:], _SWAP16)
                nc.vector.tensor_mul(p1[:], raw[:], csA_sb[:, col])
                nc.vector.tensor_mul(qs_t[:], qs_t[:], csB_sb[:, col])
                nc.vector.scalar_tensor_tensor(
                    dest[:, col], qs_t[:], sign_sb[:], p1[:],
                    mybir.AluOpType.mult, mybir.AluOpType.add)

            # ---------------- Phase 1: QKV projections --------------------
            with (
                tc.tile_pool(name="mps", bufs=6, space="PSUM") as mps,
                tc.tile_pool(name="trps", bufs=1, space="PSUM") as trps,
                tc.tile_pool(name="wsb", bufs=1) as wpool,
            ):
                w_sb = wpool.tile([DH, NDCH * FQKV], BF16)

                def emit_sblock(sb_i):
                    col = slice(sb_i * 512, (sb_i + 1) * 512)
                    ps = [mps.tile([DH, 512], F32, name=f"ps{f}", tag="ps")
                          for f in range(6)]
                    for d in range(NDCH):
                        if sb_i == 0:
                            # weight loads ride the ACT HWDGE ring, concurrent
                            # with the x loads on the SP ring
                            nc.scalar.dma_start(
                                w_sb[:, d * FQKV:(d + 1) * FQKV],
                                wqkvT[d * DH:(d + 1) * DH, :])
                        xt = xpool.tile([DH, 512], BF16, name="xt", tag="xt")
                        # split the x stream across HWDGE (sync) and SWDGE
                        # (gpsimd)
                        xeng = nc.sync if d % 2 == 0 else nc.gpsimd
                        xeng.dma_start(xt[:], xT[d * DH:(d + 1) * DH, col])
                        if sb_i == 0 and d == 8:
                            nc.sync.dma_start(csA_sb[:], csA_d[:])
                            nc.sync.dma_start(csB_sb[:], csB_d[:])
                            nc.sync.dma_start(sign_sb[:], sign_d[:])
                        if sb_i == 0 and d == 16:
                            nc.sync.dma_start(tri512_sb[:], tri512_d[:])
                            nc.sync.dma_start(ident_sb[:], ident_d[:])
                            nc.sync.dma_start(ones128_sb[:], ones128_d[:])
                        if sb_i in (1, 2) and d in (4, 20):
                            # wo (4MB bf16) rides the ACT ring during QKV
                            # (free after s-block 0's weight loads)
                            ch = 2 * (sb_i - 1) + (0 if d == 4 else 1)
                            nc.scalar.dma_start(
                                wo_sb[:, ch * DIM:(ch + 1) * DIM],
                                woT[ch * DH:(ch + 1) * DH, :])
                        for f in range(6):
                            nc.tensor.matmul(
                                ps[f][:],
                                w_sb[:, d * FQKV + f * DH: d * FQKV + (f + 1) * DH],
                                xt[:], start=(d == 0), stop=(d == NDCH - 1))
                    # Fast raw PSUM->SBUF evictions (alternating ACT/DVE) free
                    # the accumulator banks quickly; RoPE runs later from SBUF.
                    vt_t = vtp.tile([DH, 512], BF16, name="vt_t", tag="vt")
                    nc.scalar.copy(vt_t[:], ps[5][:])
                    raws = {}
                    for i, f in enumerate([0, 4, 2, 1, 3]):
                        raw = rawpool.tile([DH, 512], BF16, name="raw", tag="raw")
                        raws[f] = raw
                        if i % 2 == 1:
                            nc.scalar.copy(raw[:], ps[f][:])
                        else:
                            nc.vector.tensor_copy(raw[:], ps[f][:])
                    for t in range(4):
                        tp = trps.tile([DH, DH], BF16, name="tp", tag="tp")
                        nc.tensor.transpose(tp[:], vt_t[:, t * DH:(t + 1) * DH],
                                            ident_sb[:])
                        j = sb_i * 4 + t
                        nc.vector.tensor_copy(vn[:, j * DH:(j + 1) * DH], tp[:])
                    return raws

                for sb_i in range(NQB):
                    raws = emit_sblock(sb_i)
                    if sb_i < NQB - 1:
                        for f in [0, 4, 1, 2, 3]:
                            emit_rope(f, sb_i, raws[f])

            # ---------------- Phase 2: attention + out-proj pipelined ------
            with (
                tc.tile_pool(name="spps", bufs=3, space="PSUM") as spps,
                tc.tile_pool(name="otps", bufs=2, space="PSUM") as otps,
                tc.tile_pool(name="pwps", bufs=1, space="PSUM") as pwps,
                tc.tile_pool(name="esb", bufs=4) as epool,
                tc.tile_pool(name="essb", bufs=2) as espool,
                tc.tile_pool(name="bcsb", bufs=2) as bcpool,
                tc.tile_pool(name="evsb", bufs=3) as evpool,
            ):
                class OpjEmitter:
                    """Output projection for s-tiles of q-block bprev, emitted
                    one matmul per step() so the attention emitter can pace it.
                    Unit = (st, dh_i, jj): 8 matmuls (4 heads x 2 adjacent
                    512-wide dout cols, stationary otn[h] shared), 2 PSUM
                    evictions (ACT/DVE), one [128,1024] store. Plain state
                    machine (not a generator): tile-pool allocs from a
                    suspended generator frame break the pool's scope-matched
                    reuse dependencies."""

                    def __init__(self, bprev, u0):
                        self.units = [(st, dh_i, jj)
                                      for st in range(4 * bprev, 4 * bprev + 4)
                                      for dh_i in range(2) for jj in range(2)]
                        self.ui = 0
                        self.mi = 0
                        self.u = u0
                        self.pw0 = self.pw1 = None

                    def step(self):
                        if self.ui >= len(self.units):
                            return False
                        st, dh_i, jj = self.units[self.ui]
                        scol = slice(st * DH, (st + 1) * DH)
                        base = dh_i * 2048 + jj * 1024
                        if self.mi == 0:
                            self.pw0 = pwps.tile([DH, 512], F32, name="pw0",
                                                 tag="pw0", bufs=2)
                            self.pw1 = pwps.tile([DH, 512], F32, name="pw1",
                                                 tag="pw1", bufs=1)
                        h2, j2 = divmod(self.mi, 2)
                        o = h2 * DIM + base + j2 * 512
                        pw = self.pw0 if j2 == 0 else self.pw1
                        nc.tensor.matmul(
                            pw[:], otn[h2][:, scol], wo_sb[:, o:o + 512],
                            start=(h2 == 0), stop=(h2 == HPC - 1))
                        self.mi += 1
                        if self.mi == 8:
                            ev = evpool.tile([DH, 1024], F32, name="ev",
                                             tag="ev")
                            if self.u % 2 == 0:
                                nc.scalar.copy(ev[:, 0:512], self.pw0[:])
                                nc.vector.tensor_copy(ev[:, 512:1024],
                                                      self.pw1[:])
                            else:
                                nc.vector.tensor_copy(ev[:, 0:512], self.pw0[:])
                                nc.scalar.copy(ev[:, 512:1024], self.pw1[:])
                            dst = out_d[scol, base:base + 1024]
                            (nc.sync if self.u % 2 == 0 else nc.gpsimd
                             ).dma_start(dst, ev[:])
                            self.u += 1
                            self.mi = 0
                            self.ui += 1
                        return True

                def emit_block(h, b, fill):
                    cb = slice(b * 512, (b + 1) * 512)
                    nk = 4 * b + 4  # k tiles contributing to this q block
                    ot_b = otps.tile([DH, 512], F32, name="ot", tag="ot")
                    esum = espool.tile([DH, 512], BF16, name="esum", tag="es")
                    e_tiles = [None] * nk

                    def emit_scores(k):
                        e = epool.tile([DH, 512], BF16, name="E", tag="E")
                        e_tiles[k] = e
                        sp = spps.tile([DH, 512], F32, name="sp", tag="sp")
                        nc.tensor.matmul(
                            sp[:], kt[:, k * DH:(k + 1) * DH],
                            qt[h][:, cb], start=True, stop=True)
                        nc.scalar.activation(e[:], sp[:], EXP)
                        if k // 4 == b:
                            # diagonal block: zero the disallowed prefix
                            w = (k % 4 + 1) * DH
                            nc.vector.tensor_mul(
                                e[:, :w], e[:, :w], tri512_sb[:, 512 - w:])
                        # accumulate the softmax denominator on the DVE
                        if k == 0:
                            nc.vector.tensor_copy(esum[:], e[:])
                        else:
                            nc.vector.tensor_add(esum[:], esum[:], e[:])

                    def emit_pv(k):
                        e = e_tiles[k]
                        st_, sp_ = (k == 0), (k == nk - 1)
                        nc.tensor.matmul(ot_b[:], vn[:, k * DH:(k + 1) * DH],
                                         e[:], start=st_, stop=sp_)

                    # 2-deep software pipeline: scores run two steps ahead of
                    # PV so exp/mask latency never stalls the PE; out-proj
                    # matmuls of block b-1 are interleaved to absorb ACT lag
                    emit_scores(0)
                    if nk > 1:
                        emit_scores(1)
                    fill(1)
                    for k in range(2, nk):
                        emit_scores(k)
                        emit_pv(k - 2)
                        fill(1)
                    if nk > 1:
                        emit_pv(nk - 2)
                    emit_pv(nk - 1)
                    fill(2)

                    # denominator: one ones-matmul reduces esum across
                    # partitions (every PSUM row = colsum -> broadcast for
                    # free), then fast reciprocal + fused normalize, all off
                    # the next block's critical path.
                    dn_b = spps.tile([DH, 512], F32, name="dn", tag="sp")
                    nc.tensor.matmul(dn_b[:], ones128_sb[:], esum[:],
                                     start=True, stop=True)
                    bc_sb = bcpool.tile([DH, 512], F32, name="bc_sb", tag="bcs")
                    nc.vector.reciprocal_approx_fast(out=bc_sb[:], in_=dn_b[:])
                    nc.vector.tensor_mul(otn[h][:, cb], ot_b[:], bc_sb[:])

                gen = None

                def fill(n):
                    if gen is None:
                        return
                    for _ in range(n):
                        if not gen.step():
                            break

                # last s-block's RoPE: kt and qt[0] first (needed by b<=3 /
                # b=3 of head 0), the rest spread between early blocks
                emit_rope(4, 3, raws[4])
                emit_rope(0, 3, raws[0])
                for b in range(NQB):
                    gen = OpjEmitter(b - 1, 16 * (b - 1)) if b >= 1 else None
                    for h in range(HPC):
                        if b == 0 and h >= 1:
                            emit_rope(h, 3, raws[h])
                        emit_block(h, b, fill)
                    fill(1 << 30)  # drain the rest of block b-1's out-proj
                gen = OpjEmitter(NQB - 1, 16 * (NQB - 1))
                fill(1 << 30)  # PE-dense tail

    nc.compile()
    return nc


def get_program():
    global _PROGRAM
    if _PROGRAM is None:
        _PROGRAM = _build_program()
    return _PROGRAM


def make_in_maps(inputs):
    """Host-side sharding / layout prep. Returns one input dict per core."""
    import ml_dtypes
    bf16 = ml_dtypes.bfloat16

    x = np.asarray(inputs["x"], dtype=np.float32)
    wq = np.asarray(inputs["wq"], dtype=np.float32)
    wk = np.asarray(inputs["wk"], dtype=np.float32)
    wv = np.asarray(inputs["wv"], dtype=np.float32)
    wo = np.asarray(inputs["wo"], dtype=np.float32)
    cos = np.asarray(inputs["freqs_cos"], dtype=np.float32)  # (S, 64)
    sin = np.asarray(inputs["freqs_sin"], dtype=np.float32)

    xT = np.ascontiguousarray(x.reshape(S, DIM).T).astype(bf16)  # (DIM, S)

    perm = _head_perm()
    sq = np.float32(DH ** -0.25)  # sqrt of 1/sqrt(head_dim), folded into Q and K
    rows = np.arange(DH)
    pair_idx = 16 * (rows // 32) + (rows % 32) % 16
    csA = np.ascontiguousarray(cos.T[pair_idx] * sq).astype(bf16)   # (128, S)
    csB = np.ascontiguousarray(sin.T[pair_idx] * sq).astype(bf16)
    sign = np.where((rows % 32) < 16, -1.0, 1.0).astype(np.float32).reshape(DH, 1)
    tri = np.triu(np.ones((DH, DH), dtype=np.float32))
    tri512 = np.concatenate([np.zeros((DH, 512 - DH), np.float32), tri],
                            axis=1).astype(bf16)
    ident = np.eye(DH, dtype=np.float32).astype(bf16)
    ones128 = np.ones((DH, DH), dtype=np.float32).astype(bf16)

    wqh = wq.reshape(N_HEADS, DH, DIM)[:, perm, :]
    wkh = wk.reshape(N_KV, DH, DIM)[:, perm, :]
    wvh = wv.reshape(N_KV, DH, DIM)

    in_maps = []
    for c in range(NCORES):
        w_c = np.concatenate(
            [wqh[HPC * c:HPC * (c + 1)].reshape(HPC * DH, DIM),
             wkh[c], wvh[c]], 0)  # (768, DIM)
        wqkvT = np.ascontiguousarray(w_c.T).astype(bf16)  # (DIM, 768)
        woT = np.ascontiguousarray(
            wo[:, HPC * DH * c:HPC * DH * (c + 1)].T).astype(bf16)
        in_maps.append({
            "xT": xT, "wqkvT": wqkvT, "woT": woT,
            "csA": csA, "csB": csB, "sign": sign, "tri512": tri512,
            "ident": ident, "ones128": ones128,
        })
    return in_maps


def _ensure_ntff_hook():
    """The agent image's `antenv` lacks `axon_hooks`; recreate it so
    run_bass_kernel_spmd(trace=True) can capture NTFF profiles."""
    import sys
    try:
        from antenv.axon_hooks import get_axon_ntff_profile_hook  # noqa: F401
        return
    except ImportError:
        pass
    import contextlib
    import ctypes
    import types

    so_path = "/opt/axon/libaxon_pjrt.so"
    hook = None
    try:
        lib = ctypes.CDLL(so_path)
        if hasattr(lib, "axon_start_nrt_profile"):
            lib.axon_start_nrt_profile.argtypes = [
                ctypes.POINTER(ctypes.c_int64), ctypes.c_size_t]
            lib.axon_start_nrt_profile.restype = ctypes.c_int64
            lib.axon_stop_nrt_profile.argtypes = [ctypes.c_char_p]
            lib.axon_stop_nrt_profile.restype = ctypes.c_int64

            @contextlib.contextmanager
            def _hook(output_dir, device_ids):
                import jax
                jax.devices()
                if device_ids:
                    ids = (ctypes.c_int64 * len(device_ids))(*device_ids)
                    rc = lib.axon_start_nrt_profile(ids, len(device_ids))
                else:
                    rc = lib.axon_start_nrt_profile(None, 0)
                if rc != 0:
                    raise RuntimeError(f"axon_start_nrt_profile rc={rc}")
                try:
                    yield
                finally:
                    n = lib.axon_stop_nrt_profile(str(output_dir).encode())
                    print(f"profile: {n} file(s) written to {output_dir}")

            hook = _hook
    except OSError:
        pass

    mod = types.ModuleType("antenv.axon_hooks")
    mod._hook = hook
    mod.get_axon_ntff_profile_hook = lambda: mod._hook
    mod.set_axon_ntff_profile_hook = lambda h: setattr(mod, "_hook", h)
    sys.modules["antenv.axon_hooks"] = mod


def run(inputs, trace=False):
    from concourse.bass_utils import run_bass_kernel_spmd
    if trace:
        _ensure_ntff_hook()
    nc = get_program()
    in_maps = make_in_maps(inputs)
    res = run_bass_kernel_spmd(nc, in_maps, core_ids=list(range(NCORES)),
                               trace=trace)
    acc = np.zeros((S, DIM), dtype=np.float32)
    for r in res.results:
        acc += np.asarray(r["out"], dtype=np.float32)
    return acc.reshape(B, S, DIM), res


def kernel(**inputs):
    out, _ = run(inputs, trace=False)
    return out


# revision 10
# speedup vs baseline: 1.2461x; 1.0139x over previous
"""Mistral-style MHA prefill kernel for Trainium2, 8-way tensor-parallel over heads.

Problem (hardcoded): B=1, S=2048, DIM=4096, 32 q-heads / 8 kv-heads, head_dim=128,
sliding window 2048 (== S, so the mask is exactly causal), rope theta 1e4.

Sharding: core c owns q-heads [4c, 4c+4) and kv-head c. wq/wk/wv are sharded on the
head axis, wo on its input (head) axis; each core computes a full-shape partial
output and the host sums the 8 partials (row-parallel linear + host all-reduce).

v2 design (vs the fp32r baseline at ~520 us):
  - All matmul operands are bf16 (PSUM accumulation stays fp32). Same PE rate as
    float32r (1 cyc/row) but halves DMA traffic, so s-block 0 is no longer
    DMA-bound, and unlocks the 2x/4x DVE perf modes for 2-byte dtypes.
    Numpy-simulated end-to-end rel err of the full-bf16 scheme: 4.4e-3.
  - The 160 softmax-denominator matmuls are off the PE: e-tiles accumulate
    elementwise into esum on the DVE (bf16, 2x mode), and ONE ones-matmul per
    (head, block) reduces esum across partitions (broadcast for free).
  - reciprocal() [3.4 us!] -> reciprocal_approx_fast() [~0.7 us, 18 bits].
  - With the denominator matmuls gone, attention is ACT-bound (exp = 674 ns/tile
    vs 426 ns of PE work). So the output projection of q-block b-1 is software-
    pipelined INTO the attention stream of q-block b: its matmuls (no exp
    dependency) fill the PE while ACT catches up on exps. Block order is b-outer,
    head-inner; out-proj of block 3 forms a PE-dense tail.
  - Layouts as baseline: x pre-transposed, per-quadrant rope permutation with
    stream_shuffle +-16, sqrt(scale) folded into the rope tables, transposed
    scores S_T[k, q], causality at (k-tile, 512-q-block) granularity, diagonal
    masked with a zeros|triangle tile.
"""

import numpy as np

B = 1
S = 2048
DIM = 4096
N_HEADS = 32
N_KV = 8
DH = 128
NCORES = 8
HPC = N_HEADS // NCORES  # q heads per core
FQKV = HPC * DH + 2 * DH  # 768 projection rows per core
NKT = S // DH  # 16 k tiles
NQB = S // 512  # 4 q blocks
NDCH = DIM // DH  # 32 contraction chunks

_PROGRAM = None

# stream_shuffle mask: swap 16-partition halves within each 32-partition quadrant
_SWAP16 = [(i + 16) % 32 for i in range(32)]


def _head_perm():
    """Permutation of head_dim rows: quadrant q holds [re_16q..re_16q+15,
    im_16q..im_16q+15], so RoPE pairs are +-16 apart within a quadrant."""
    p = np.empty(DH, dtype=np.int64)
    for row in range(DH):
        q, j = divmod(row, 32)
        i = 16 * q + (j % 16)  # rope pair index
        p[row] = 2 * i + (0 if j < 16 else 1)
    return p


def _build_program():
    import concourse.bacc as bacc
    import concourse.mybir as mybir
    import concourse.tile as tile

    F32 = mybir.dt.float32
    BF16 = mybir.dt.bfloat16
    EXP = mybir.ActivationFunctionType.Exp

    nc = bacc.Bacc("TRN2", target_bir_lowering=False, debug=False,
                   enable_asserts=False)

    xT = nc.dram_tensor("xT", [DIM, S], BF16, kind="ExternalInput")
    wqkvT = nc.dram_tensor("wqkvT", [DIM, FQKV], BF16, kind="ExternalInput")
    woT = nc.dram_tensor("woT", [HPC * DH, DIM], BF16, kind="ExternalInput")
    csA_d = nc.dram_tensor("csA", [DH, S], BF16, kind="ExternalInput")
    csB_d = nc.dram_tensor("csB", [DH, S], BF16, kind="ExternalInput")
    sign_d = nc.dram_tensor("sign", [DH, 1], F32, kind="ExternalInput")
    tri512_d = nc.dram_tensor("tri512", [DH, 512], BF16, kind="ExternalInput")
    ident_d = nc.dram_tensor("ident", [DH, DH], BF16, kind="ExternalInput")
    ones128_d = nc.dram_tensor("ones128", [DH, DH], BF16, kind="ExternalInput")
    out_d = nc.dram_tensor("out", [S, DIM], F32, kind="ExternalOutput")

    with tile.TileContext(nc) as tc:
        with (
            tc.tile_pool(name="consts", bufs=1) as cpool,
            tc.tile_pool(name="persist", bufs=1) as ppool,
            tc.tile_pool(name="xin", bufs=8) as xpool,
            tc.tile_pool(name="ropet", bufs=2) as rtp,
            tc.tile_pool(name="rawsb", bufs=5) as rawpool,
            tc.tile_pool(name="vtt", bufs=1) as vtp,
        ):
            csA_sb = cpool.tile([DH, S], BF16)
            csB_sb = cpool.tile([DH, S], BF16)
            sign_sb = cpool.tile([DH, 1], F32)
            tri512_sb = cpool.tile([DH, 512], BF16)
            ident_sb = cpool.tile([DH, DH], BF16)
            ones128_sb = cpool.tile([DH, DH], BF16)

            qt = [ppool.tile([DH, S], BF16, name=f"qt{h}") for h in range(HPC)]
            kt = ppool.tile([DH, S], BF16)
            vn = ppool.tile([DH, S], BF16)  # V normal layout, 16 [128,128] chunks
            wo_sb = ppool.tile([DH, HPC * DIM], BF16)
            # otn aliases qt: attention block b is the last reader of
            # qt[h][:, b*512:(b+1)*512], so the normalized out^T overwrites it.
            otn = qt

            def emit_rope(f, sb_i, raw):
                # head_dim permuted so pairs sit +-16 apart within each
                # 32-partition quadrant: dest = p1 + sign*p3 where
                # p1 = q*cos, p3 = halfswap(q)*sin. All bf16 SBUF operands so
                # the muls run 2x and the stt 4x on the DVE.
                col = slice(sb_i * 512, (sb_i + 1) * 512)
                dest = qt[f] if f < HPC else kt
                qs_t = rtp.tile([DH, 512], BF16, name="qs_t", tag="qs")
                p1 = rtp.tile([DH, 512], BF16, name="p1", tag="p1")
                nc.vector.stream_shuffle(qs_t[:], raw[:], _SWAP16)
                nc.vector.tensor_mul(p1[:], raw[:], csA_sb[:, col])
                nc.vector.tensor_mul(qs_t[:], qs_t[:], csB_sb[:, col])
                nc.vector.scalar_tensor_tensor(
                    dest[:, col], qs_t[:], sign_sb[:], p1[:],
                    mybir.AluOpType.mult, mybir.AluOpType.add)

            # ---------------- Phase 1: QKV projections --------------------
            with (
                tc.tile_pool(name="mps", bufs=6, space="PSUM") as mps,
                tc.tile_pool(name="trps", bufs=1, space="PSUM") as trps,
                tc.tile_pool(name="wsb", bufs=1) as wpool,
            ):
                w_sb = wpool.tile([DH, NDCH * FQKV], BF16)

                def emit_sblock(sb_i):
                    col = slice(sb_i * 512, (sb_i + 1) * 512)
                    ps = [mps.tile([DH, 512], F32, name=f"ps{f}", tag="ps")
                          for f in range(6)]
                    for d in range(NDCH):
                        if sb_i == 0:
                            # weight chunks alternate between the two HWDGE
                            # rings (ACT/SP) so the w stream outpaces compute;
                            # x rides the SWDGE (gpsimd) ring meanwhile
                            weng = nc.scalar if d % 2 == 0 else nc.sync
                            weng.dma_start(
                                w_sb[:, d * FQKV:(d + 1) * FQKV],
                                wqkvT[d * DH:(d + 1) * DH, :])
                        xt = xpool.tile([DH, 512], BF16, name="xt", tag="xt")
                        if sb_i == 0:
                            xeng = nc.sync if d % 4 == 3 else nc.gpsimd
                        else:
                            xeng = nc.sync if d % 2 == 0 else nc.gpsimd
                        xeng.dma_start(xt[:], xT[d * DH:(d + 1) * DH, col])
                        if sb_i == 0 and d == 8:
                            nc.scalar.dma_start(csA_sb[:], csA_d[:])
                            nc.scalar.dma_start(csB_sb[:], csB_d[:])
                            nc.scalar.dma_start(sign_sb[:], sign_d[:])
                        if sb_i == 0 and d == 16:
                            nc.scalar.dma_start(tri512_sb[:], tri512_d[:])
                            nc.scalar.dma_start(ident_sb[:], ident_d[:])
                            nc.scalar.dma_start(ones128_sb[:], ones128_d[:])
                        if sb_i in (1, 2) and d in (4, 20):
                            # wo (4MB bf16) rides the ACT ring during QKV
                            # (free after s-block 0's weight loads)
                            ch = 2 * (sb_i - 1) + (0 if d == 4 else 1)
                            nc.scalar.dma_start(
                                wo_sb[:, ch * DIM:(ch + 1) * DIM],
                                woT[ch * DH:(ch + 1) * DH, :])
                        for f in range(6):
                            nc.tensor.matmul(
                                ps[f][:],
                                w_sb[:, d * FQKV + f * DH: d * FQKV + (f + 1) * DH],
                                xt[:], start=(d == 0), stop=(d == NDCH - 1))
                    # Fast raw PSUM->SBUF evictions (alternating ACT/DVE) free
                    # the accumulator banks quickly; RoPE runs later from SBUF.
                    vt_t = vtp.tile([DH, 512], BF16, name="vt_t", tag="vt")
                    nc.scalar.copy(vt_t[:], ps[5][:])
                    raws = {}
                    for i, f in enumerate([0, 4, 2, 1, 3]):
                        raw = rawpool.tile([DH, 512], BF16, name="raw", tag="raw")
                        raws[f] = raw
                        if i % 2 == 1:
                            nc.scalar.copy(raw[:], ps[f][:])
                        else:
                            nc.vector.tensor_copy(raw[:], ps[f][:])
                    for t in range(4):
                        tp = trps.tile([DH, DH], BF16, name="tp", tag="tp")
                        nc.tensor.transpose(tp[:], vt_t[:, t * DH:(t + 1) * DH],
                                            ident_sb[:])
                        j = sb_i * 4 + t
                        nc.vector.tensor_copy(vn[:, j * DH:(j + 1) * DH], tp[:])
                    return raws

                for sb_i in range(NQB):
                    raws = emit_sblock(sb_i)
                    if sb_i < NQB - 1:
                        for f in [0, 4, 1, 2, 3]:
                            emit_rope(f, sb_i, raws[f])

            # ---------------- Phase 2: attention + out-proj pipelined ------
            with (
                tc.tile_pool(name="spps", bufs=3, space="PSUM") as spps,
                tc.tile_pool(name="otps", bufs=2, space="PSUM") as otps,
                tc.tile_pool(name="pwps", bufs=1, space="PSUM") as pwps,
                tc.tile_pool(name="esb", bufs=4) as epool,
                tc.tile_pool(name="essb", bufs=2) as espool,
                tc.tile_pool(name="bcsb", bufs=2) as bcpool,
                tc.tile_pool(name="evsb", bufs=3) as evpool,
            ):
                class OpjEmitter:
                    """Output projection for s-tiles of q-block bprev, emitted
                    one matmul per step() so the attention emitter can pace it.
                    Unit = (st, dh_i, jj): 8 matmuls (4 heads x 2 adjacent
                    512-wide dout cols, stationary otn[h] shared), 2 PSUM
                    evictions (ACT/DVE), one [128,1024] store. Plain state
                    machine (not a generator): tile-pool allocs from a
                    suspended generator frame break the pool's scope-matched
                    reuse dependencies."""

                    def __init__(self, bprev, u0):
                        self.units = [(st, dh_i, jj)
                                      for st in range(4 * bprev, 4 * bprev + 4)
                                      for dh_i in range(2) for jj in range(2)]
                        self.ui = 0
                        self.mi = 0
                        self.u = u0
                        self.pw0 = self.pw1 = None

                    def step(self):
                        if self.ui >= len(self.units):
                            return False
                        st, dh_i, jj = self.units[self.ui]
                        scol = slice(st * DH, (st + 1) * DH)
                        base = dh_i * 2048 + jj * 1024
                        if self.mi == 0:
                            self.pw0 = pwps.tile([DH, 512], F32, name="pw0",
                                                 tag="pw0", bufs=2)
                            self.pw1 = pwps.tile([DH, 512], F32, name="pw1",
                                                 tag="pw1", bufs=1)
                        h2, j2 = divmod(self.mi, 2)
                        o = h2 * DIM + base + j2 * 512
                        pw = self.pw0 if j2 == 0 else self.pw1
                        nc.tensor.matmul(
                            pw[:], otn[h2][:, scol], wo_sb[:, o:o + 512],
                            start=(h2 == 0), stop=(h2 == HPC - 1))
                        self.mi += 1
                        if self.mi == 8:
                            ev = evpool.tile([DH, 1024], F32, name="ev",
                                             tag="ev")
                            if self.u % 2 == 0:
                                nc.scalar.copy(ev[:, 0:512], self.pw0[:])
                                nc.vector.tensor_copy(ev[:, 512:1024],
                                                      self.pw1[:])
                            else:
                                nc.vector.tensor_copy(ev[:, 0:512], self.pw0[:])
                                nc.scalar.copy(ev[:, 512:1024], self.pw1[:])
                            dst = out_d[scol, base:base + 1024]
                            if self.ui >= len(self.units) - 2:
                                # split the final stores across both HWDGE
                                # rings to shrink the kernel tail
                                nc.sync.dma_start(
                                    out_d[scol, base:base + 512], ev[:, 0:512])
                                nc.scalar.dma_start(
                                    out_d[scol, base + 512:base + 1024],
                                    ev[:, 512:1024])
                            else:
                                # stores alternate the two HWDGE rings; the
                                # SWDGE (gpsimd) ring is too slow for stores
                                (nc.sync if self.u % 2 == 0 else nc.scalar
                                 ).dma_start(dst, ev[:])
                            self.u += 1
                            self.mi = 0
                            self.ui += 1
                        return True

                def emit_block(h, b, fill):
                    cb = slice(b * 512, (b + 1) * 512)
                    nk = 4 * b + 4  # k tiles contributing to this q block
                    ot_b = otps.tile([DH, 512], F32, name="ot", tag="ot")
                    esum = espool.tile([DH, 512], BF16, name="esum", tag="es")
                    e_tiles = [None] * nk

                    def emit_scores(k):
                        e = epool.tile([DH, 512], BF16, name="E", tag="E")
                        e_tiles[k] = e
                        sp = spps.tile([DH, 512], F32, name="sp", tag="sp")
                        nc.tensor.matmul(
                            sp[:], kt[:, k * DH:(k + 1) * DH],
                            qt[h][:, cb], start=True, stop=True)
                        nc.scalar.activation(e[:], sp[:], EXP)
                        if k // 4 == b:
                            # diagonal block: zero the disallowed prefix
                            w = (k % 4 + 1) * DH
                            nc.vector.tensor_mul(
                                e[:, :w], e[:, :w], tri512_sb[:, 512 - w:])
                        # accumulate the softmax denominator on the DVE
                        if k == 0:
                            nc.vector.tensor_copy(esum[:], e[:])
                        else:
                            nc.vector.tensor_add(esum[:], esum[:], e[:])

                    def emit_pv(k):
                        e = e_tiles[k]
                        st_, sp_ = (k == 0), (k == nk - 1)
                        nc.tensor.matmul(ot_b[:], vn[:, k * DH:(k + 1) * DH],
                                         e[:], start=st_, stop=sp_)

                    # 2-deep software pipeline: scores run two steps ahead of
                    # PV so exp/mask latency never stalls the PE; out-proj
                    # matmuls of block b-1 are interleaved to absorb ACT lag
                    emit_scores(0)
                    if nk > 1:
                        emit_scores(1)
                    fill(1)
                    for k in range(2, nk):
                        emit_scores(k)
                        emit_pv(k - 2)
                        fill(1)
                    if nk > 1:
                        emit_pv(nk - 2)
                    emit_pv(nk - 1)
                    fill(2)

                    # denominator: one ones-matmul reduces esum across
                    # partitions (every PSUM row = colsum -> broadcast for
                    # free), then fast reciprocal + fused normalize, all off
                    # the next block's critical path.
                    dn_b = spps.tile([DH, 512], F32, name="dn", tag="sp")
                    nc.tensor.matmul(dn_b[:], ones128_sb[:], esum[:],
                                     start=True, stop=True)
                    bc_sb = bcpool.tile([DH, 512], F32, name="bc_sb", tag="bcs")
                    nc.vector.reciprocal_approx_fast(out=bc_sb[:], in_=dn_b[:])
                    nc.vector.tensor_mul(otn[h][:, cb], ot_b[:], bc_sb[:])

                gen = None

                def fill(n):
                    if gen is None:
                        return
                    for _ in range(n):
                        if not gen.step():
                            break

                # last s-block's RoPE: kt and qt[0] first (needed by b<=3 /
                # b=3 of head 0), the rest spread between early blocks
                emit_rope(4, 3, raws[4])
                emit_rope(0, 3, raws[0])
                for b in range(NQB):
                    gen = OpjEmitter(b - 1, 16 * (b - 1)) if b >= 1 else None
                    for h in range(HPC):
                        if b == 0 and h >= 1:
                            emit_rope(h, 3, raws[h])
                        emit_block(h, b, fill)
                    fill(1 << 30)  # drain the rest of block b-1's out-proj
                gen = OpjEmitter(NQB - 1, 16 * (NQB - 1))
                fill(1 << 30)  # PE-dense tail

    nc.compile()
    return nc


def get_program():
    global _PROGRAM
    if _PROGRAM is None:
        _PROGRAM = _build_program()
    return _PROGRAM


def make_in_maps(inputs):
    """Host-side sharding / layout prep. Returns one input dict per core."""
    import ml_dtypes
    bf16 = ml_dtypes.bfloat16

    x = np.asarray(inputs["x"], dtype=np.float32)
    wq = np.asarray(inputs["wq"], dtype=np.float32)
    wk = np.asarray(inputs["wk"], dtype=np.float32)
    wv = np.asarray(inputs["wv"], dtype=np.float32)
    wo = np.asarray(inputs["wo"], dtype=np.float32)
    cos = np.asarray(inputs["freqs_cos"], dtype=np.float32)  # (S, 64)
    sin = np.asarray(inputs["freqs_sin"], dtype=np.float32)

    xT = np.ascontiguousarray(x.reshape(S, DIM).T).astype(bf16)  # (DIM, S)

    perm = _head_perm()
    sq = np.float32(DH ** -0.25)  # sqrt of 1/sqrt(head_dim), folded into Q and K
    rows = np.arange(DH)
    pair_idx = 16 * (rows // 32) + (rows % 32) % 16
    csA = np.ascontiguousarray(cos.T[pair_idx] * sq).astype(bf16)   # (128, S)
    csB = np.ascontiguousarray(sin.T[pair_idx] * sq).astype(bf16)
    sign = np.where((rows % 32) < 16, -1.0, 1.0).astype(np.float32).reshape(DH, 1)
    tri = np.triu(np.ones((DH, DH), dtype=np.float32))
    tri512 = np.concatenate([np.zeros((DH, 512 - DH), np.float32), tri],
                            axis=1).astype(bf16)
    ident = np.eye(DH, dtype=np.float32).astype(bf16)
    ones128 = np.ones((DH, DH), dtype=np.float32).astype(bf16)

    wqh = wq.reshape(N_HEADS, DH, DIM)[:, perm, :]
    wkh = wk.reshape(N_KV, DH, DIM)[:, perm, :]
    wvh = wv.reshape(N_KV, DH, DIM)

    in_maps = []
    for c in range(NCORES):
        w_c = np.concatenate(
            [wqh[HPC * c:HPC * (c + 1)].reshape(HPC * DH, DIM),
             wkh[c], wvh[c]], 0)  # (768, DIM)
        wqkvT = np.ascontiguousarray(w_c.T).astype(bf16)  # (DIM, 768)
        woT = np.ascontiguousarray(
            wo[:, HPC * DH * c:HPC * DH * (c + 1)].T).astype(bf16)
        in_maps.append({
            "xT": xT, "wqkvT": wqkvT, "woT": woT,
            "csA": csA, "csB": csB, "sign": sign, "tri512": tri512,
            "ident": ident, "ones128": ones128,
        })
    return in_maps


def _ensure_ntff_hook():
    """The agent image's `antenv` lacks `axon_hooks`; recreate it so
    run_bass_kernel_spmd(trace=True) can capture NTFF profiles."""
    import sys
    try:
        from antenv.axon_hooks import get_axon_ntff_profile_hook  # noqa: F401
        return
    except ImportError:
        pass
    import contextlib
    import ctypes
    import types

    so_path = "/opt/axon/libaxon_pjrt.so"
    hook = None
    try:
        lib = ctypes.CDLL(so_path)
        if hasattr(lib, "axon_start_nrt_profile"):
            lib.axon_start_nrt_profile.argtypes = [
                ctypes.POINTER(ctypes.c_int64), ctypes.c_size_t]
            lib.axon_start_nrt_profile.restype = ctypes.c_int64
            lib.axon_stop_nrt_profile.argtypes = [ctypes.c_char_p]
            lib.axon_stop_nrt_profile.restype = ctypes.c_int64

            @contextlib.contextmanager
            def _hook(output_dir, device_ids):
                import jax
                jax.devices()
                if device_ids:
                    ids = (ctypes.c_int64 * len(device_ids))(*device_ids)
                    rc = lib.axon_start_nrt_profile(ids, len(device_ids))
                else:
                    rc = lib.axon_start_nrt_profile(None, 0)
                if rc != 0:
                    raise RuntimeError(f"axon_start_nrt_profile rc={rc}")
                try:
                    yield
                finally:
                    n = lib.axon_stop_nrt_profile(str(output_dir).encode())
                    print(f"profile: {n} file(s) written to {output_dir}")

            hook = _hook
    except OSError:
        pass

    mod = types.ModuleType("antenv.axon_hooks")
    mod._hook = hook
    mod.get_axon_ntff_profile_hook = lambda: mod._hook
    mod.set_axon_ntff_profile_hook = lambda h: setattr(mod, "_hook", h)
    sys.modules["antenv.axon_hooks"] = mod


def run(inputs, trace=False):
    from concourse.bass_utils import run_bass_kernel_spmd
    if trace:
        _ensure_ntff_hook()
    nc = get_program()
    in_maps = make_in_maps(inputs)
    res = run_bass_kernel_spmd(nc, in_maps, core_ids=list(range(NCORES)),
                               trace=trace)
    acc = np.zeros((S, DIM), dtype=np.float32)
    for r in res.results:
        acc += np.asarray(r["out"], dtype=np.float32)
    return acc.reshape(B, S, DIM), res


def kernel(**inputs):
    out, _ = run(inputs, trace=False)
    return out


# revision 13
# speedup vs baseline: 1.2511x; 1.0040x over previous
"""Mistral-style MHA prefill kernel for Trainium2, 8-way tensor-parallel over heads.

Problem (hardcoded): B=1, S=2048, DIM=4096, 32 q-heads / 8 kv-heads, head_dim=128,
sliding window 2048 (== S, so the mask is exactly causal), rope theta 1e4.

Sharding: core c owns q-heads [4c, 4c+4) and kv-head c. wq/wk/wv are sharded on the
head axis, wo on its input (head) axis; each core computes a full-shape partial
output and the host sums the 8 partials (row-parallel linear + host all-reduce).

v2 design (vs the fp32r baseline at ~520 us):
  - All matmul operands are bf16 (PSUM accumulation stays fp32). Same PE rate as
    float32r (1 cyc/row) but halves DMA traffic, so s-block 0 is no longer
    DMA-bound, and unlocks the 2x/4x DVE perf modes for 2-byte dtypes.
    Numpy-simulated end-to-end rel err of the full-bf16 scheme: 4.4e-3.
  - The 160 softmax-denominator matmuls are off the PE: e-tiles accumulate
    elementwise into esum on the DVE (bf16, 2x mode), and ONE ones-matmul per
    (head, block) reduces esum across partitions (broadcast for free).
  - reciprocal() [3.4 us!] -> reciprocal_approx_fast() [~0.7 us, 18 bits].
  - With the denominator matmuls gone, attention is ACT-bound (exp = 674 ns/tile
    vs 426 ns of PE work). So the output projection of q-block b-1 is software-
    pipelined INTO the attention stream of q-block b: its matmuls (no exp
    dependency) fill the PE while ACT catches up on exps. Block order is b-outer,
    head-inner; out-proj of block 3 forms a PE-dense tail.
  - Layouts as baseline: x pre-transposed, per-quadrant rope permutation with
    stream_shuffle +-16, sqrt(scale) folded into the rope tables, transposed
    scores S_T[k, q], causality at (k-tile, 512-q-block) granularity, diagonal
    masked with a zeros|triangle tile.
"""

import numpy as np

B = 1
S = 2048
DIM = 4096
N_HEADS = 32
N_KV = 8
DH = 128
NCORES = 8
HPC = N_HEADS // NCORES  # q heads per core
FQKV = HPC * DH + 2 * DH  # 768 projection rows per core
NKT = S // DH  # 16 k tiles
NQB = S // 512  # 4 q blocks
NDCH = DIM // DH  # 32 contraction chunks

_PROGRAM = None

# stream_shuffle mask: swap 16-partition halves within each 32-partition quadrant
_SWAP16 = [(i + 16) % 32 for i in range(32)]


def _head_perm():
    """Permutation of head_dim rows: quadrant q holds [re_16q..re_16q+15,
    im_16q..im_16q+15], so RoPE pairs are +-16 apart within a quadrant."""
    p = np.empty(DH, dtype=np.int64)
    for row in range(DH):
        q, j = divmod(row, 32)
        i = 16 * q + (j % 16)  # rope pair index
        p[row] = 2 * i + (0 if j < 16 else 1)
    return p


def _build_program():
    import concourse.bacc as bacc
    import concourse.mybir as mybir
    import concourse.tile as tile

    F32 = mybir.dt.float32
    BF16 = mybir.dt.bfloat16
    EXP = mybir.ActivationFunctionType.Exp

    nc = bacc.Bacc("TRN2", target_bir_lowering=False, debug=False,
                   enable_asserts=False)

    xT = nc.dram_tensor("xT", [DIM, S], BF16, kind="ExternalInput")
    wqkvT = nc.dram_tensor("wqkvT", [DIM, FQKV], BF16, kind="ExternalInput")
    woT = nc.dram_tensor("woT", [HPC * DH, DIM], BF16, kind="ExternalInput")
    csA_d = nc.dram_tensor("csA", [DH, S], BF16, kind="ExternalInput")
    csB_d = nc.dram_tensor("csB", [DH, S], BF16, kind="ExternalInput")
    sign_d = nc.dram_tensor("sign", [DH, 1], F32, kind="ExternalInput")
    tri512_d = nc.dram_tensor("tri512", [DH, 512], BF16, kind="ExternalInput")
    ident_d = nc.dram_tensor("ident", [DH, DH], BF16, kind="ExternalInput")
    ones128_d = nc.dram_tensor("ones128", [DH, DH], BF16, kind="ExternalInput")
    out_d = nc.dram_tensor("out", [S, DIM], F32, kind="ExternalOutput")

    with tile.TileContext(nc) as tc:
        with (
            tc.tile_pool(name="consts", bufs=1) as cpool,
            tc.tile_pool(name="persist", bufs=1) as ppool,
            tc.tile_pool(name="xin", bufs=8) as xpool,
            tc.tile_pool(name="ropet", bufs=2) as rtp,
            tc.tile_pool(name="rawsb", bufs=5) as rawpool,
            tc.tile_pool(name="vtt", bufs=1) as vtp,
        ):
            csA_sb = cpool.tile([DH, S], BF16)
            csB_sb = cpool.tile([DH, S], BF16)
            sign_sb = cpool.tile([DH, 1], F32)
            tri512_sb = cpool.tile([DH, 512], BF16)
            ident_sb = cpool.tile([DH, DH], BF16)
            ones128_sb = cpool.tile([DH, DH], BF16)

            qt = [ppool.tile([DH, S], BF16, name=f"qt{h}") for h in range(HPC)]
            kt = ppool.tile([DH, S], BF16)
            vn = ppool.tile([DH, S], BF16)  # V normal layout, 16 [128,128] chunks
            wo_sb = ppool.tile([DH, HPC * DIM], BF16)
            # otn aliases qt: attention block b is the last reader of
            # qt[h][:, b*512:(b+1)*512], so the normalized out^T overwrites it.
            otn = qt

            def emit_rope(f, sb_i, raw):
                # head_dim permuted so pairs sit +-16 apart within each
                # 32-partition quadrant: dest = p1 + sign*p3 where
                # p1 = q*cos, p3 = halfswap(q)*sin. All bf16 SBUF operands so
                # the muls run 2x and the stt 4x on the DVE.
                col = slice(sb_i * 512, (sb_i + 1) * 512)
                dest = qt[f] if f < HPC else kt
                qs_t = rtp.tile([DH, 512], BF16, name="qs_t", tag="qs")
                p1 = rtp.tile([DH, 512], BF16, name="p1", tag="p1")
                nc.vector.stream_shuffle(qs_t[:], raw[:], _SWAP16)
                nc.vector.tensor_mul(p1[:], raw[:], csA_sb[:, col])
                nc.vector.tensor_mul(qs_t[:], qs_t[:], csB_sb[:, col])
                nc.vector.scalar_tensor_tensor(
                    dest[:, col], qs_t[:], sign_sb[:], p1[:],
                    mybir.AluOpType.mult, mybir.AluOpType.add)

            # ---------------- Phase 1: QKV projections --------------------
            with (
                tc.tile_pool(name="mps", bufs=6, space="PSUM") as mps,
                tc.tile_pool(name="trps", bufs=1, space="PSUM") as trps,
                tc.tile_pool(name="wsb", bufs=1) as wpool,
            ):
                w_sb = wpool.tile([DH, NDCH * FQKV], BF16)

                def emit_sblock(sb_i):
                    col = slice(sb_i * 512, (sb_i + 1) * 512)
                    ps = [mps.tile([DH, 512], F32, name=f"ps{f}", tag="ps")
                          for f in range(6)]
                    for d in range(NDCH):
                        if sb_i == 0:
                            # w stream mostly on the ACT ring (every 8th chunk
                            # on SP for slack); first chunk split so the very
                            # first stationary lands ASAP
                            if d == 0:
                                nc.scalar.dma_start(
                                    w_sb[:, 0:DH], wqkvT[0:DH, 0:DH])
                                nc.sync.dma_start(
                                    w_sb[:, DH:FQKV], wqkvT[0:DH, DH:FQKV])
                            else:
                                weng = nc.sync if d % 8 == 7 else nc.scalar
                                weng.dma_start(
                                    w_sb[:, d * FQKV:(d + 1) * FQKV],
                                    wqkvT[d * DH:(d + 1) * DH, :])
                        xt = xpool.tile([DH, 512], BF16, name="xt", tag="xt")
                        # split the x stream across HWDGE (sync) and SWDGE
                        # (gpsimd)
                        xeng = nc.sync if d % 2 == 1 else nc.gpsimd
                        xeng.dma_start(xt[:], xT[d * DH:(d + 1) * DH, col])
                        if sb_i == 0 and d == 12:
                            nc.sync.dma_start(csA_sb[:], csA_d[:])
                            nc.sync.dma_start(csB_sb[:], csB_d[:])
                            nc.sync.dma_start(sign_sb[:], sign_d[:])
                        if sb_i == 0 and d == 20:
                            nc.sync.dma_start(tri512_sb[:], tri512_d[:])
                            nc.sync.dma_start(ident_sb[:], ident_d[:])
                            nc.sync.dma_start(ones128_sb[:], ones128_d[:])
                        if sb_i in (1, 2) and d in (4, 20):
                            # wo (4MB bf16) rides the ACT ring during QKV
                            # (free after s-block 0's weight loads)
                            ch = 2 * (sb_i - 1) + (0 if d == 4 else 1)
                            nc.scalar.dma_start(
                                wo_sb[:, ch * DIM:(ch + 1) * DIM],
                                woT[ch * DH:(ch + 1) * DH, :])
                        for f in range(6):
                            nc.tensor.matmul(
                                ps[f][:],
                                w_sb[:, d * FQKV + f * DH: d * FQKV + (f + 1) * DH],
                                xt[:], start=(d == 0), stop=(d == NDCH - 1))
                    # Fast raw PSUM->SBUF evictions (alternating ACT/DVE) free
                    # the accumulator banks quickly; RoPE runs later from SBUF.
                    vt_t = vtp.tile([DH, 512], BF16, name="vt_t", tag="vt")
                    nc.scalar.copy(vt_t[:], ps[5][:])
                    raws = {}
                    for i, f in enumerate([0, 4, 2, 1, 3]):
                        raw = rawpool.tile([DH, 512], BF16, name="raw", tag="raw")
                        raws[f] = raw
                        if i % 2 == 1:
                            nc.scalar.copy(raw[:], ps[f][:])
                        else:
                            nc.vector.tensor_copy(raw[:], ps[f][:])
                    for t in range(4):
                        tp = trps.tile([DH, DH], BF16, name="tp", tag="tp")
                        nc.tensor.transpose(tp[:], vt_t[:, t * DH:(t + 1) * DH],
                                            ident_sb[:])
                        j = sb_i * 4 + t
                        nc.vector.tensor_copy(vn[:, j * DH:(j + 1) * DH], tp[:])
                    return raws

                for sb_i in range(NQB):
                    raws = emit_sblock(sb_i)
                    if sb_i < NQB - 1:
                        for f in [0, 4, 1, 2, 3]:
                            emit_rope(f, sb_i, raws[f])

            # ---------------- Phase 2: attention + out-proj pipelined ------
            with (
                tc.tile_pool(name="spps", bufs=3, space="PSUM") as spps,
                tc.tile_pool(name="otps", bufs=2, space="PSUM") as otps,
                tc.tile_pool(name="pwps", bufs=1, space="PSUM") as pwps,
                tc.tile_pool(name="esb", bufs=4) as epool,
                tc.tile_pool(name="essb", bufs=2) as espool,
                tc.tile_pool(name="bcsb", bufs=2) as bcpool,
                tc.tile_pool(name="evsb", bufs=3) as evpool,
            ):
                class OpjEmitter:
                    """Output projection for s-tiles of q-block bprev, emitted
                    one matmul per step() so the attention emitter can pace it.
                    Unit = (st, dh_i, jj): 8 matmuls (4 heads x 2 adjacent
                    512-wide dout cols, stationary otn[h] shared), 2 PSUM
                    evictions (ACT/DVE), one [128,1024] store. Plain state
                    machine (not a generator): tile-pool allocs from a
                    suspended generator frame break the pool's scope-matched
                    reuse dependencies."""

                    def __init__(self, bprev, u0):
                        self.units = [(st, dh_i, jj)
                                      for st in range(4 * bprev, 4 * bprev + 4)
                                      for dh_i in range(2) for jj in range(2)]
                        self.ui = 0
                        self.mi = 0
                        self.u = u0
                        self.pw0 = self.pw1 = None

                    def step(self):
                        if self.ui >= len(self.units):
                            return False
                        st, dh_i, jj = self.units[self.ui]
                        scol = slice(st * DH, (st + 1) * DH)
                        base = dh_i * 2048 + jj * 1024
                        if self.mi == 0:
                            self.pw0 = pwps.tile([DH, 512], F32, name="pw0",
                                                 tag="pw0", bufs=2)
                            self.pw1 = pwps.tile([DH, 512], F32, name="pw1",
                                                 tag="pw1", bufs=1)
                        # j-major: pw0's accumulation (4 matmuls) completes
                        # first and evicts while pw1's matmuls run; pw1's
                        # next-unit reuse then trails its eviction by 4
                        # matmuls, so a single pw1 buffer never stalls the PE
                        j2, h2 = divmod(self.mi, HPC)
                        o = h2 * DIM + base + j2 * 512
                        pw = self.pw0 if j2 == 0 else self.pw1
                        nc.tensor.matmul(
                            pw[:], otn[h2][:, scol], wo_sb[:, o:o + 512],
                            start=(h2 == 0), stop=(h2 == HPC - 1))
                        self.mi += 1
                        if self.mi == HPC:
                            self.ev = evpool.tile([DH, 1024], F32, name="ev",
                                                  tag="ev")
                            if self.u % 2 == 0:
                                nc.scalar.copy(self.ev[:, 0:512], self.pw0[:])
                            else:
                                nc.vector.tensor_copy(self.ev[:, 0:512],
                                                      self.pw0[:])
                        if self.mi == 8:
                            ev = self.ev
                            if self.u % 2 == 0:
                                nc.vector.tensor_copy(ev[:, 512:1024],
                                                      self.pw1[:])
                            else:
                                nc.scalar.copy(ev[:, 512:1024], self.pw1[:])
                            dst = out_d[scol, base:base + 1024]
                            if self.ui >= len(self.units) - 4:
                                # split the final stores across both HWDGE
                                # rings to shrink the kernel tail
                                nc.sync.dma_start(
                                    out_d[scol, base:base + 512], ev[:, 0:512])
                                nc.scalar.dma_start(
                                    out_d[scol, base + 512:base + 1024],
                                    ev[:, 512:1024])
                            else:
                                # stores alternate the two HWDGE rings; the
                                # SWDGE (gpsimd) ring is too slow for stores
                                (nc.sync if self.u % 2 == 0 else nc.scalar
                                 ).dma_start(dst, ev[:])
                            self.u += 1
                            self.mi = 0
                            self.ui += 1
                        return True

                def emit_block(h, b, fill):
                    cb = slice(b * 512, (b + 1) * 512)
                    nk = 4 * b + 4  # k tiles contributing to this q block
                    ot_b = otps.tile([DH, 512], F32, name="ot", tag="ot")
                    esum = espool.tile([DH, 512], BF16, name="esum", tag="es")
                    e_tiles = [None] * nk

                    def emit_scores(k):
                        e = epool.tile([DH, 512], BF16, name="E", tag="E")
                        e_tiles[k] = e
                        sp = spps.tile([DH, 512], F32, name="sp", tag="sp")
                        nc.tensor.matmul(
                            sp[:], kt[:, k * DH:(k + 1) * DH],
                            qt[h][:, cb], start=True, stop=True)
                        nc.scalar.activation(e[:], sp[:], EXP)
                        if k // 4 == b:
                            # diagonal block: zero the disallowed prefix
                            w = (k % 4 + 1) * DH
                            nc.vector.tensor_mul(
                                e[:, :w], e[:, :w], tri512_sb[:, 512 - w:])
                        # accumulate the softmax denominator on the DVE
                        if k == 0:
                            nc.vector.tensor_copy(esum[:], e[:])
                        else:
                            nc.vector.tensor_add(esum[:], esum[:], e[:])

                    def emit_pv(k):
                        e = e_tiles[k]
                        st_, sp_ = (k == 0), (k == nk - 1)
                        nc.tensor.matmul(ot_b[:], vn[:, k * DH:(k + 1) * DH],
                                         e[:], start=st_, stop=sp_)

                    # 2-deep software pipeline: scores run two steps ahead of
                    # PV so exp/mask latency never stalls the PE; out-proj
                    # matmuls of block b-1 are interleaved to absorb ACT lag
                    emit_scores(0)
                    if nk > 1:
                        emit_scores(1)
                    fill(1)
                    for k in range(2, nk):
                        emit_scores(k)
                        emit_pv(k - 2)
                        fill(1)
                    if nk > 1:
                        emit_pv(nk - 2)
                    emit_pv(nk - 1)
                    fill(2)

                    # denominator: one ones-matmul reduces esum across
                    # partitions (every PSUM row = colsum -> broadcast for
                    # free), then fast reciprocal + fused normalize, all off
                    # the next block's critical path.
                    dn_b = spps.tile([DH, 512], F32, name="dn", tag="sp")
                    nc.tensor.matmul(dn_b[:], ones128_sb[:], esum[:],
                                     start=True, stop=True)
                    bc_sb = bcpool.tile([DH, 512], F32, name="bc_sb", tag="bcs")
                    nc.vector.reciprocal_approx_fast(out=bc_sb[:], in_=dn_b[:])
                    nc.vector.tensor_mul(otn[h][:, cb], ot_b[:], bc_sb[:])

                gen = None

                def fill(n):
                    if gen is None:
                        return
                    for _ in range(n):
                        if not gen.step():
                            break

                # last s-block's RoPE: kt and qt[0] first (needed by b<=3 /
                # b=3 of head 0), the rest spread between early blocks
                emit_rope(4, 3, raws[4])
                emit_rope(0, 3, raws[0])
                for b in range(NQB):
                    gen = OpjEmitter(b - 1, 16 * (b - 1)) if b >= 1 else None
                    for h in range(HPC):
                        if b == 0 and h >= 1:
                            emit_rope(h, 3, raws[h])
                        emit_block(h, b, fill)
                    fill(1 << 30)  # drain the rest of block b-1's out-proj
                gen = OpjEmitter(NQB - 1, 16 * (NQB - 1))
                fill(1 << 30)  # PE-dense tail

    nc.compile()
    return nc


def get_program():
    global _PROGRAM
    if _PROGRAM is None:
        _PROGRAM = _build_program()
    return _PROGRAM


def make_in_maps(inputs):
    """Host-side sharding / layout prep. Returns one input dict per core."""
    import ml_dtypes
    bf16 = ml_dtypes.bfloat16

    x = np.asarray(inputs["x"], dtype=np.float32)
    wq = np.asarray(inputs["wq"], dtype=np.float32)
    wk = np.asarray(inputs["wk"], dtype=np.float32)
    wv = np.asarray(inputs["wv"], dtype=np.float32)
    wo = np.asarray(inputs["wo"], dtype=np.float32)
    cos = np.asarray(inputs["freqs_cos"], dtype=np.float32)  # (S, 64)
    sin = np.asarray(inputs["freqs_sin"], dtype=np.float32)

    xT = np.ascontiguousarray(x.reshape(S, DIM).T).astype(bf16)  # (DIM, S)

    perm = _head_perm()
    sq = np.float32(DH ** -0.25)  # sqrt of 1/sqrt(head_dim), folded into Q and K
    rows = np.arange(DH)
    pair_idx = 16 * (rows // 32) + (rows % 32) % 16
    csA = np.ascontiguousarray(cos.T[pair_idx] * sq).astype(bf16)   # (128, S)
    csB = np.ascontiguousarray(sin.T[pair_idx] * sq).astype(bf16)
    sign = np.where((rows % 32) < 16, -1.0, 1.0).astype(np.float32).reshape(DH, 1)
    tri = np.triu(np.ones((DH, DH), dtype=np.float32))
    tri512 = np.concatenate([np.zeros((DH, 512 - DH), np.float32), tri],
                            axis=1).astype(bf16)
    ident = np.eye(DH, dtype=np.float32).astype(bf16)
    ones128 = np.ones((DH, DH), dtype=np.float32).astype(bf16)

    wqh = wq.reshape(N_HEADS, DH, DIM)[:, perm, :]
    wkh = wk.reshape(N_KV, DH, DIM)[:, perm, :]
    wvh = wv.reshape(N_KV, DH, DIM)

    in_maps = []
    for c in range(NCORES):
        w_c = np.concatenate(
            [wqh[HPC * c:HPC * (c + 1)].reshape(HPC * DH, DIM),
             wkh[c], wvh[c]], 0)  # (768, DIM)
        wqkvT = np.ascontiguousarray(w_c.T).astype(bf16)  # (DIM, 768)
        woT = np.ascontiguousarray(
            wo[:, HPC * DH * c:HPC * DH * (c + 1)].T).astype(bf16)
        in_maps.append({
            "xT": xT, "wqkvT": wqkvT, "woT": woT,
            "csA": csA, "csB": csB, "sign": sign, "tri512": tri512,
            "ident": ident, "ones128": ones128,
        })
    return in_maps


def _ensure_ntff_hook():
    """The agent image's `antenv` lacks `axon_hooks`; recreate it so
    run_bass_kernel_spmd(trace=True) can capture NTFF profiles."""
    import sys
    try:
        from antenv.axon_hooks import get_axon_ntff_profile_hook  # noqa: F401
        return
    except ImportError:
        pass
    import contextlib
    import ctypes
    import types

    so_path = "/opt/axon/libaxon_pjrt.so"
    hook = None
    try:
        lib = ctypes.CDLL(so_path)
        if hasattr(lib, "axon_start_nrt_profile"):
            lib.axon_start_nrt_profile.argtypes = [
                ctypes.POINTER(ctypes.c_int64), ctypes.c_size_t]
            lib.axon_start_nrt_profile.restype = ctypes.c_int64
            lib.axon_stop_nrt_profile.argtypes = [ctypes.c_char_p]
            lib.axon_stop_nrt_profile.restype = ctypes.c_int64

            @contextlib.contextmanager
            def _hook(output_dir, device_ids):
                import jax
                jax.devices()
                if device_ids:
                    ids = (ctypes.c_int64 * len(device_ids))(*device_ids)
                    rc = lib.axon_start_nrt_profile(ids, len(device_ids))
                else:
                    rc = lib.axon_start_nrt_profile(None, 0)
                if rc != 0:
                    raise RuntimeError(f"axon_start_nrt_profile rc={rc}")
                try:
                    yield
                finally:
                    n = lib.axon_stop_nrt_profile(str(output_dir).encode())
                    print(f"profile: {n} file(s) written to {output_dir}")

            hook = _hook
    except OSError:
        pass

    mod = types.ModuleType("antenv.axon_hooks")
    mod._hook = hook
    mod.get_axon_ntff_profile_hook = lambda: mod._hook
    mod.set_axon_ntff_profile_hook = lambda h: setattr(mod, "_hook", h)
    sys.modules["antenv.axon_hooks"] = mod


def run(inputs, trace=False):
    from concourse.bass_utils import run_bass_kernel_spmd
    if trace:
        _ensure_ntff_hook()
    nc = get_program()
    in_maps = make_in_maps(inputs)
    res = run_bass_kernel_spmd(nc, in_maps, core_ids=list(range(NCORES)),
                               trace=trace)
    acc = np.zeros((S, DIM), dtype=np.float32)
    for r in res.results:
        acc += np.asarray(r["out"], dtype=np.float32)
    return acc.reshape(B, S, DIM), res


def kernel(**inputs):
    out, _ = run(inputs, trace=False)
    return out


# revision 18
# speedup vs baseline: 1.2736x; 1.0180x over previous
"""Mistral-style MHA prefill kernel for Trainium2, 8-way tensor-parallel over heads.

Problem (hardcoded): B=1, S=2048, DIM=4096, 32 q-heads / 8 kv-heads, head_dim=128,
sliding window 2048 (== S, so the mask is exactly causal), rope theta 1e4.

Sharding: core c owns q-heads [4c, 4c+4) and kv-head c. wq/wk/wv are sharded on the
head axis, wo on its input (head) axis; each core computes a full-shape partial
output and the host sums the 8 partials (row-parallel linear + host all-reduce).

v2 design (vs the fp32r baseline at ~520 us):
  - All matmul operands are bf16 (PSUM accumulation stays fp32). Same PE rate as
    float32r (1 cyc/row) but halves DMA traffic, so s-block 0 is no longer
    DMA-bound, and unlocks the 2x/4x DVE perf modes for 2-byte dtypes.
    Numpy-simulated end-to-end rel err of the full-bf16 scheme: 4.4e-3.
  - The 160 softmax-denominator matmuls are off the PE: e-tiles accumulate
    elementwise into esum on the DVE (bf16, 2x mode), and ONE ones-matmul per
    (head, block) reduces esum across partitions (broadcast for free).
  - reciprocal() [3.4 us!] -> reciprocal_approx_fast() [~0.7 us, 18 bits].
  - With the denominator matmuls gone, attention is ACT-bound (exp = 674 ns/tile
    vs 426 ns of PE work). So the output projection of q-block b-1 is software-
    pipelined INTO the attention stream of q-block b: its matmuls (no exp
    dependency) fill the PE while ACT catches up on exps. Block order is b-outer,
    head-inner; out-proj of block 3 forms a PE-dense tail.
  - Layouts as baseline: x pre-transposed, per-quadrant rope permutation with
    stream_shuffle +-16, sqrt(scale) folded into the rope tables, transposed
    scores S_T[k, q], causality at (k-tile, 512-q-block) granularity, diagonal
    masked with a zeros|triangle tile.
"""

import numpy as np

B = 1
S = 2048
DIM = 4096
N_HEADS = 32
N_KV = 8
DH = 128
NCORES = 8
HPC = N_HEADS // NCORES  # q heads per core
FQKV = HPC * DH + 2 * DH  # 768 projection rows per core
NKT = S // DH  # 16 k tiles
NQB = S // 512  # 4 q blocks
NDCH = DIM // DH  # 32 contraction chunks

_PROGRAM = None

# stream_shuffle mask: swap 16-partition halves within each 32-partition quadrant
_SWAP16 = [(i + 16) % 32 for i in range(32)]


def _head_perm():
    """Permutation of head_dim rows: quadrant q holds [re_16q..re_16q+15,
    im_16q..im_16q+15], so RoPE pairs are +-16 apart within a quadrant."""
    p = np.empty(DH, dtype=np.int64)
    for row in range(DH):
        q, j = divmod(row, 32)
        i = 16 * q + (j % 16)  # rope pair index
        p[row] = 2 * i + (0 if j < 16 else 1)
    return p


def _build_program():
    import concourse.bacc as bacc
    import concourse.mybir as mybir
    import concourse.tile as tile

    F32 = mybir.dt.float32
    BF16 = mybir.dt.bfloat16
    EXP = mybir.ActivationFunctionType.Exp

    nc = bacc.Bacc("TRN2", target_bir_lowering=False, debug=False,
                   enable_asserts=False)

    # All inputs are pre-shuffled on the host into the EXACT SBUF layout
    # ([128 partitions, free]) so every load is one huge contiguous-row DMA:
    # per-transfer fixed overhead (~2.8 us) and per-queue issue pace
    # (~1.4 us/transfer) dominate small transfers, so few+big wins.
    x2_d = nc.dram_tensor("x2", [DH, NQB * NDCH * 512], BF16,
                          kind="ExternalInput")
    wq2_d = nc.dram_tensor("wq2", [DH, NDCH * FQKV], BF16,
                           kind="ExternalInput")
    wo2_d = nc.dram_tensor("wo2", [DH, HPC * DIM], BF16, kind="ExternalInput")
    # consts pack: csA | csB | tri512 | ident | ones128
    CPK = 2 * S + 512 + 2 * DH
    cpk_d = nc.dram_tensor("cpk", [DH, CPK], BF16, kind="ExternalInput")
    sign_d = nc.dram_tensor("sign", [DH, 1], F32, kind="ExternalInput")
    out_d = nc.dram_tensor("out", [S, DIM], F32, kind="ExternalOutput")

    with tile.TileContext(nc) as tc:
        with (
            tc.tile_pool(name="consts", bufs=1) as cpool,
            tc.tile_pool(name="persist", bufs=1) as ppool,
            tc.tile_pool(name="xin", bufs=3) as xpool,
            tc.tile_pool(name="ropet", bufs=2) as rtp,
            tc.tile_pool(name="rawsb", bufs=5) as rawpool,
            tc.tile_pool(name="vtt", bufs=1) as vtp,
        ):
            cpk_sb = cpool.tile([DH, CPK], BF16)
            csA_sb = cpk_sb[:, 0:S]
            csB_sb = cpk_sb[:, S:2 * S]
            tri512_sb = cpk_sb[:, 2 * S:2 * S + 512]
            ident_sb = cpk_sb[:, 2 * S + 512:2 * S + 512 + DH]
            ones128_sb = cpk_sb[:, 2 * S + 512 + DH:2 * S + 512 + 2 * DH]
            sign_sb = cpool.tile([DH, 1], F32)

            qt = [ppool.tile([DH, S], BF16, name=f"qt{h}") for h in range(HPC)]
            kt = ppool.tile([DH, S], BF16)
            vn = ppool.tile([DH, S], BF16)  # V normal layout, 16 [128,128] chunks
            wo_sb = ppool.tile([DH, HPC * DIM], BF16)
            # otn aliases qt: attention block b is the last reader of
            # qt[h][:, b*512:(b+1)*512], so the normalized out^T overwrites it.
            otn = qt

            def emit_rope(f, sb_i, raw):
                # head_dim permuted so pairs sit +-16 apart within each
                # 32-partition quadrant: dest = p1 + sign*p3 where
                # p1 = q*cos, p3 = halfswap(q)*sin. All bf16 SBUF operands so
                # the muls run 2x and the stt 4x on the DVE.
                col = slice(sb_i * 512, (sb_i + 1) * 512)
                dest = qt[f] if f < HPC else kt
                qs_t = rtp.tile([DH, 512], BF16, name="qs_t", tag="qs")
                p1 = rtp.tile([DH, 512], BF16, name="p1", tag="p1")
                nc.vector.stream_shuffle(qs_t[:], raw[:], _SWAP16)
                nc.vector.tensor_mul(p1[:], raw[:], csA_sb[:, col])
                nc.vector.tensor_mul(qs_t[:], qs_t[:], csB_sb[:, col])
                nc.vector.scalar_tensor_tensor(
                    dest[:, col], qs_t[:], sign_sb[:], p1[:],
                    mybir.AluOpType.mult, mybir.AluOpType.add)

            # ---------------- Phase 1: QKV projections --------------------
            with (
                tc.tile_pool(name="mps", bufs=6, space="PSUM") as mps,
                tc.tile_pool(name="trps", bufs=1, space="PSUM") as trps,
                tc.tile_pool(name="wsb", bufs=1) as wpool,
            ):
                w_sb = wpool.tile([DH, NDCH * FQKV], BF16)

                def emit_sblock(sb_i):
                    # x consumed in multi-chunk groups: one DMA per group
                    # (contiguous in the host-shuffled x2 layout), alternating
                    # the SP-HWDGE and SWDGE rings. s-block 0 leads with a
                    # small group so the first matmul starts ASAP, and its
                    # w groups stream on the ACT ring in matching sizes.
                    groups = ([(0, 2), (2, 8), (8, 16), (16, 24), (24, 32)]
                              if sb_i == 0 else
                              [(0, 8), (8, 16), (16, 24), (24, 32)])
                    ps = [mps.tile([DH, 512], F32, name=f"ps{f}", tag="ps")
                          for f in range(6)]
                    for gi, (a, b) in enumerate(groups):
                        xg = xpool.tile([DH, (b - a) * 512], BF16, name="xg",
                                        tag="xg")
                        xeng = nc.sync if (sb_i + gi) % 2 == 0 else nc.gpsimd
                        xo = sb_i * NDCH * 512
                        xeng.dma_start(xg[:],
                                       x2_d[:, xo + a * 512:xo + b * 512])
                        if sb_i == 0:
                            nc.scalar.dma_start(
                                w_sb[:, a * FQKV:b * FQKV],
                                wq2_d[:, a * FQKV:b * FQKV])
                            if gi == len(groups) - 1:
                                nc.scalar.dma_start(cpk_sb[:], cpk_d[:])
                                nc.scalar.dma_start(sign_sb[:], sign_d[:])
                        if sb_i == 1 and gi < 2:
                            # wo (4MB bf16) in two halves on the ACT ring,
                            # after s-block 0's bus traffic has drained
                            half = HPC * DIM // 2
                            nc.scalar.dma_start(
                                wo_sb[:, gi * half:(gi + 1) * half],
                                wo2_d[:, gi * half:(gi + 1) * half])
                        for d in range(a, b):
                            xcol = slice((d - a) * 512, (d - a + 1) * 512)
                            for f in range(6):
                                nc.tensor.matmul(
                                    ps[f][:],
                                    w_sb[:, d * FQKV + f * DH:
                                         d * FQKV + (f + 1) * DH],
                                    xg[:, xcol], start=(d == 0),
                                    stop=(d == NDCH - 1))
                    # Fast raw PSUM->SBUF evictions (alternating ACT/DVE) free
                    # the accumulator banks quickly; RoPE runs later from SBUF.
                    vt_t = vtp.tile([DH, 512], BF16, name="vt_t", tag="vt")
                    nc.scalar.copy(vt_t[:], ps[5][:])
                    raws = {}
                    for i, f in enumerate([0, 4, 2, 1, 3]):
                        raw = rawpool.tile([DH, 512], BF16, name="raw", tag="raw")
                        raws[f] = raw
                        if i % 2 == 1:
                            nc.scalar.copy(raw[:], ps[f][:])
                        else:
                            nc.vector.tensor_copy(raw[:], ps[f][:])
                    for t in range(4):
                        tp = trps.tile([DH, DH], BF16, name="tp", tag="tp")
                        nc.tensor.transpose(tp[:], vt_t[:, t * DH:(t + 1) * DH],
                                            ident_sb[:])
                        j = sb_i * 4 + t
                        nc.vector.tensor_copy(vn[:, j * DH:(j + 1) * DH], tp[:])
                    return raws

                for sb_i in range(NQB):
                    raws = emit_sblock(sb_i)
                    if sb_i < NQB - 1:
                        for f in [0, 4, 1, 2, 3]:
                            emit_rope(f, sb_i, raws[f])

            # ---------------- Phase 2: attention + out-proj pipelined ------
            with (
                tc.tile_pool(name="spps", bufs=3, space="PSUM") as spps,
                tc.tile_pool(name="otps", bufs=2, space="PSUM") as otps,
                tc.tile_pool(name="pwps", bufs=1, space="PSUM") as pwps,
                tc.tile_pool(name="esb", bufs=4) as epool,
                tc.tile_pool(name="essb", bufs=2) as espool,
                tc.tile_pool(name="bcsb", bufs=2) as bcpool,
                tc.tile_pool(name="evsb", bufs=3) as evpool,
            ):
                class OpjEmitter:
                    """Output projection for s-tiles of q-block bprev, emitted
                    one matmul per step() so the attention emitter can pace it.
                    Unit = (st, dh_i, jj): 8 matmuls (4 heads x 2 adjacent
                    512-wide dout cols, stationary otn[h] shared), 2 PSUM
                    evictions (ACT/DVE), one [128,1024] store. Plain state
                    machine (not a generator): tile-pool allocs from a
                    suspended generator frame break the pool's scope-matched
                    reuse dependencies."""

                    def __init__(self, bprev, u0):
                        self.units = [(st, dh_i, jj)
                                      for st in range(4 * bprev, 4 * bprev + 4)
                                      for dh_i in range(2) for jj in range(2)]
                        self.ui = 0
                        self.mi = 0
                        self.u = u0
                        self.pw0 = self.pw1 = None

                    def step(self):
                        if self.ui >= len(self.units):
                            return False
                        st, dh_i, jj = self.units[self.ui]
                        scol = slice(st * DH, (st + 1) * DH)
                        base = dh_i * 2048 + jj * 1024
                        if self.mi == 0:
                            self.pw0 = pwps.tile([DH, 512], F32, name="pw0",
                                                 tag="pw0", bufs=2)
                            self.pw1 = pwps.tile([DH, 512], F32, name="pw1",
                                                 tag="pw1", bufs=1)
                        # j-major: pw0's accumulation (4 matmuls) completes
                        # first and evicts while pw1's matmuls run; pw1's
                        # next-unit reuse then trails its eviction by 4
                        # matmuls, so a single pw1 buffer never stalls the PE
                        j2, h2 = divmod(self.mi, HPC)
                        o = h2 * DIM + base + j2 * 512
                        pw = self.pw0 if j2 == 0 else self.pw1
                        nc.tensor.matmul(
                            pw[:], otn[h2][:, scol], wo_sb[:, o:o + 512],
                            start=(h2 == 0), stop=(h2 == HPC - 1))
                        self.mi += 1
                        if self.mi == HPC:
                            self.ev = evpool.tile([DH, 1024], F32, name="ev",
                                                  tag="ev")
                            if self.u % 2 == 0:
                                nc.scalar.copy(self.ev[:, 0:512], self.pw0[:])
                            else:
                                nc.vector.tensor_copy(self.ev[:, 0:512],
                                                      self.pw0[:])
                        if self.mi == 8:
                            ev = self.ev
                            if self.u % 2 == 0:
                                nc.vector.tensor_copy(ev[:, 512:1024],
                                                      self.pw1[:])
                            else:
                                nc.scalar.copy(ev[:, 512:1024], self.pw1[:])
                            dst = out_d[scol, base:base + 1024]
                            if self.ui >= len(self.units) - 4:
                                # split the final stores across both HWDGE
                                # rings to shrink the kernel tail
                                nc.sync.dma_start(
                                    out_d[scol, base:base + 512], ev[:, 0:512])
                                nc.scalar.dma_start(
                                    out_d[scol, base + 512:base + 1024],
                                    ev[:, 512:1024])
                            else:
                                # stores alternate the two HWDGE rings; the
                                # SWDGE (gpsimd) ring is too slow for stores
                                (nc.sync if self.u % 2 == 0 else nc.scalar
                                 ).dma_start(dst, ev[:])
                            self.u += 1
                            self.mi = 0
                            self.ui += 1
                        return True

                def emit_block(h, b, fill):
                    cb = slice(b * 512, (b + 1) * 512)
                    nk = 4 * b + 4  # k tiles contributing to this q block
                    ot_b = otps.tile([DH, 512], F32, name="ot", tag="ot")
                    esum = espool.tile([DH, 512], BF16, name="esum", tag="es")
                    e_tiles = [None] * nk

                    def emit_scores(k):
                        e = epool.tile([DH, 512], BF16, name="E", tag="E")
                        e_tiles[k] = e
                        sp = spps.tile([DH, 512], F32, name="sp", tag="sp")
                        nc.tensor.matmul(
                            sp[:], kt[:, k * DH:(k + 1) * DH],
                            qt[h][:, cb], start=True, stop=True)
                        nc.scalar.activation(e[:], sp[:], EXP)
                        if k // 4 == b:
                            # diagonal block: zero the disallowed prefix
                            w = (k % 4 + 1) * DH
                            nc.vector.tensor_mul(
                                e[:, :w], e[:, :w], tri512_sb[:, 512 - w:])
                        # accumulate the softmax denominator on the DVE
                        if k == 0:
                            nc.vector.tensor_copy(esum[:], e[:])
                        else:
                            nc.vector.tensor_add(esum[:], esum[:], e[:])

                    def emit_pv(k):
                        e = e_tiles[k]
                        st_, sp_ = (k == 0), (k == nk - 1)
                        nc.tensor.matmul(ot_b[:], vn[:, k * DH:(k + 1) * DH],
                                         e[:], start=st_, stop=sp_)

                    # 2-deep software pipeline: scores run two steps ahead of
                    # PV so exp/mask latency never stalls the PE; out-proj
                    # matmuls of block b-1 are interleaved to absorb ACT lag
                    emit_scores(0)
                    if nk > 1:
                        emit_scores(1)
                    fill(1)
                    for k in range(2, nk):
                        emit_scores(k)
                        emit_pv(k - 2)
                        fill(1)
                    if nk > 1:
                        emit_pv(nk - 2)
                    emit_pv(nk - 1)
                    fill(4)

                    # denominator: one ones-matmul reduces esum across
                    # partitions (every PSUM row = colsum -> broadcast for
                    # free), then fast reciprocal + fused normalize, all off
                    # the next block's critical path.
                    dn_b = spps.tile([DH, 512], F32, name="dn", tag="sp")
                    nc.tensor.matmul(dn_b[:], ones128_sb[:], esum[:],
                                     start=True, stop=True)
                    bc_sb = bcpool.tile([DH, 512], F32, name="bc_sb", tag="bcs")
                    nc.vector.reciprocal_approx_fast(out=bc_sb[:], in_=dn_b[:])
                    nc.vector.tensor_mul(otn[h][:, cb], ot_b[:], bc_sb[:])

                gen = None

                def fill(n):
                    if gen is None:
                        return
                    for _ in range(n):
                        if not gen.step():
                            break

                # last s-block's RoPE: kt and qt[0] first (needed by b<=3 /
                # b=3 of head 0), the rest spread between early blocks
                emit_rope(4, 3, raws[4])
                emit_rope(0, 3, raws[0])
                for b in range(NQB):
                    gen = OpjEmitter(b - 1, 16 * (b - 1)) if b >= 1 else None
                    for h in range(HPC):
                        if b == 0 and h >= 1:
                            emit_rope(h, 3, raws[h])
                        emit_block(h, b, fill)
                    fill(1 << 30)  # drain the rest of block b-1's out-proj
                gen = OpjEmitter(NQB - 1, 16 * (NQB - 1))
                fill(1 << 30)  # PE-dense tail

    nc.compile()
    return nc


def get_program():
    global _PROGRAM
    if _PROGRAM is None:
        _PROGRAM = _build_program()
    return _PROGRAM


def make_in_maps(inputs):
    """Host-side sharding / layout prep. Returns one input dict per core."""
    import ml_dtypes
    bf16 = ml_dtypes.bfloat16

    x = np.asarray(inputs["x"], dtype=np.float32)
    wq = np.asarray(inputs["wq"], dtype=np.float32)
    wk = np.asarray(inputs["wk"], dtype=np.float32)
    wv = np.asarray(inputs["wv"], dtype=np.float32)
    wo = np.asarray(inputs["wo"], dtype=np.float32)
    cos = np.asarray(inputs["freqs_cos"], dtype=np.float32)  # (S, 64)
    sin = np.asarray(inputs["freqs_sin"], dtype=np.float32)

    xT = x.reshape(S, DIM).T  # (DIM, S)
    # x2: SBUF-mirror layout [128, (sb, d, col)] so each x group load is one
    # contiguous-row DMA: x2[p, sb*32*512 + d*512 + c] = xT[d*128+p, sb*512+c]
    x2 = np.ascontiguousarray(
        xT.reshape(NDCH, DH, NQB, 512).transpose(1, 2, 0, 3).reshape(
            DH, NQB * NDCH * 512)).astype(bf16)

    perm = _head_perm()
    sq = np.float32(DH ** -0.25)  # sqrt of 1/sqrt(head_dim), folded into Q and K
    rows = np.arange(DH)
    pair_idx = 16 * (rows // 32) + (rows % 32) % 16
    csA = np.ascontiguousarray(cos.T[pair_idx] * sq)   # (128, S)
    csB = np.ascontiguousarray(sin.T[pair_idx] * sq)
    sign = np.where((rows % 32) < 16, -1.0, 1.0).astype(np.float32).reshape(DH, 1)
    tri = np.triu(np.ones((DH, DH), dtype=np.float32))
    tri512 = np.concatenate([np.zeros((DH, 512 - DH), np.float32), tri], axis=1)
    ident = np.eye(DH, dtype=np.float32)
    ones128 = np.ones((DH, DH), dtype=np.float32)
    # consts pack mirrors the cpk_sb slices: csA | csB | tri512 | ident | ones
    cpk = np.concatenate([csA, csB, tri512, ident, ones128],
                         axis=1).astype(bf16)

    wqh = wq.reshape(N_HEADS, DH, DIM)[:, perm, :]
    wkh = wk.reshape(N_KV, DH, DIM)[:, perm, :]
    wvh = wv.reshape(N_KV, DH, DIM)

    in_maps = []
    for c in range(NCORES):
        w_c = np.concatenate(
            [wqh[HPC * c:HPC * (c + 1)].reshape(HPC * DH, DIM),
             wkh[c], wvh[c]], 0)  # (768, DIM)
        wqkvT = w_c.T  # (DIM, 768)
        # wq2: SBUF-mirror [128, (d, f)]: wq2[p, d*768+f] = wqkvT[d*128+p, f]
        wq2 = np.ascontiguousarray(
            wqkvT.reshape(NDCH, DH, FQKV).transpose(1, 0, 2).reshape(
                DH, NDCH * FQKV)).astype(bf16)
        woT = wo[:, HPC * DH * c:HPC * DH * (c + 1)].T  # (512, DIM)
        # wo2: SBUF-mirror [128, (ch, dout)]
        wo2 = np.ascontiguousarray(
            woT.reshape(HPC, DH, DIM).transpose(1, 0, 2).reshape(
                DH, HPC * DIM)).astype(bf16)
        in_maps.append({
            "x2": x2, "wq2": wq2, "wo2": wo2, "cpk": cpk, "sign": sign,
        })
    return in_maps


def _ensure_ntff_hook():
    """The agent image's `antenv` lacks `axon_hooks`; recreate it so
    run_bass_kernel_spmd(trace=True) can capture NTFF profiles."""
    import sys
    try:
        from antenv.axon_hooks import get_axon_ntff_profile_hook  # noqa: F401
        return
    except ImportError:
        pass
    import contextlib
    import ctypes
    import types

    so_path = "/opt/axon/libaxon_pjrt.so"
    hook = None
    try:
        lib = ctypes.CDLL(so_path)
        if hasattr(lib, "axon_start_nrt_profile"):
            lib.axon_start_nrt_profile.argtypes = [
                ctypes.POINTER(ctypes.c_int64), ctypes.c_size_t]
            lib.axon_start_nrt_profile.restype = ctypes.c_int64
            lib.axon_stop_nrt_profile.argtypes = [ctypes.c_char_p]
            lib.axon_stop_nrt_profile.restype = ctypes.c_int64

            @contextlib.contextmanager
            def _hook(output_dir, device_ids):
                import jax
                jax.devices()
                if device_ids:
                    ids = (ctypes.c_int64 * len(device_ids))(*device_ids)
                    rc = lib.axon_start_nrt_profile(ids, len(device_ids))
                else:
                    rc = lib.axon_start_nrt_profile(None, 0)
                if rc != 0:
                    raise RuntimeError(f"axon_start_nrt_profile rc={rc}")
                try:
                    yield
                finally:
                    n = lib.axon_stop_nrt_profile(str(output_dir).encode())
                    print(f"profile: {n} file(s) written to {output_dir}")

            hook = _hook
    except OSError:
        pass

    mod = types.ModuleType("antenv.axon_hooks")
    mod._hook = hook
    mod.get_axon_ntff_profile_hook = lambda: mod._hook
    mod.set_axon_ntff_profile_hook = lambda h: setattr(mod, "_hook", h)
    sys.modules["antenv.axon_hooks"] = mod


def run(inputs, trace=False):
    from concourse.bass_utils import run_bass_kernel_spmd
    if trace:
        _ensure_ntff_hook()
    nc = get_program()
    in_maps = make_in_maps(inputs)
    res = run_bass_kernel_spmd(nc, in_maps, core_ids=list(range(NCORES)),
                               trace=trace)
    acc = np.zeros((S, DIM), dtype=np.float32)
    for r in res.results:
        acc += np.asarray(r["out"], dtype=np.float32)
    return acc.reshape(B, S, DIM), res


def kernel(**inputs):
    out, _ = run(inputs, trace=False)
    return out


# revision 23
# speedup vs baseline: 1.2757x; 1.0017x over previous
"""Mistral-style MHA prefill kernel for Trainium2, 8-way tensor-parallel over heads.

Problem (hardcoded): B=1, S=2048, DIM=4096, 32 q-heads / 8 kv-heads, head_dim=128,
sliding window 2048 (== S, so the mask is exactly causal), rope theta 1e4.

Sharding: core c owns q-heads [4c, 4c+4) and kv-head c. wq/wk/wv are sharded on the
head axis, wo on its input (head) axis; each core computes a full-shape partial
output and the host sums the 8 partials (row-parallel linear + host all-reduce).

v2 design (vs the fp32r baseline at ~520 us):
  - All matmul operands are bf16 (PSUM accumulation stays fp32). Same PE rate as
    float32r (1 cyc/row) but halves DMA traffic, so s-block 0 is no longer
    DMA-bound, and unlocks the 2x/4x DVE perf modes for 2-byte dtypes.
    Numpy-simulated end-to-end rel err of the full-bf16 scheme: 4.4e-3.
  - The 160 softmax-denominator matmuls are off the PE: e-tiles accumulate
    elementwise into esum on the DVE (bf16, 2x mode), and ONE ones-matmul per
    (head, block) reduces esum across partitions (broadcast for free).
  - reciprocal() [3.4 us!] -> reciprocal_approx_fast() [~0.7 us, 18 bits].
  - With the denominator matmuls gone, attention is ACT-bound (exp = 674 ns/tile
    vs 426 ns of PE work). So the output projection of q-block b-1 is software-
    pipelined INTO the attention stream of q-block b: its matmuls (no exp
    dependency) fill the PE while ACT catches up on exps. Block order is b-outer,
    head-inner; out-proj of block 3 forms a PE-dense tail.
  - Layouts as baseline: x pre-transposed, per-quadrant rope permutation with
    stream_shuffle +-16, sqrt(scale) folded into the rope tables, transposed
    scores S_T[k, q], causality at (k-tile, 512-q-block) granularity, diagonal
    masked with a zeros|triangle tile.
"""

import numpy as np

B = 1
S = 2048
DIM = 4096
N_HEADS = 32
N_KV = 8
DH = 128
NCORES = 8
HPC = N_HEADS // NCORES  # q heads per core
FQKV = HPC * DH + 2 * DH  # 768 projection rows per core
NKT = S // DH  # 16 k tiles
NQB = S // 512  # 4 q blocks
NDCH = DIM // DH  # 32 contraction chunks

_PROGRAM = None

# stream_shuffle mask: swap 16-partition halves within each 32-partition quadrant
_SWAP16 = [(i + 16) % 32 for i in range(32)]


def _head_perm():
    """Permutation of head_dim rows: quadrant q holds [re_16q..re_16q+15,
    im_16q..im_16q+15], so RoPE pairs are +-16 apart within a quadrant."""
    p = np.empty(DH, dtype=np.int64)
    for row in range(DH):
        q, j = divmod(row, 32)
        i = 16 * q + (j % 16)  # rope pair index
        p[row] = 2 * i + (0 if j < 16 else 1)
    return p


def _build_program():
    import concourse.bacc as bacc
    import concourse.mybir as mybir
    import concourse.tile as tile

    F32 = mybir.dt.float32
    BF16 = mybir.dt.bfloat16
    EXP = mybir.ActivationFunctionType.Exp

    nc = bacc.Bacc("TRN2", target_bir_lowering=False, debug=False,
                   enable_asserts=False)

    # All inputs are pre-shuffled on the host into the EXACT SBUF layout
    # ([128 partitions, free]) so every load is one huge contiguous-row DMA:
    # per-transfer fixed overhead (~2.8 us) and per-queue issue pace
    # (~1.4 us/transfer) dominate small transfers, so few+big wins.
    x2_d = nc.dram_tensor("x2", [DH, NQB * NDCH * 512], BF16,
                          kind="ExternalInput")
    wq2_d = nc.dram_tensor("wq2", [DH, NDCH * FQKV], BF16,
                           kind="ExternalInput")
    wo2_d = nc.dram_tensor("wo2", [DH, HPC * DIM], BF16, kind="ExternalInput")
    # consts pack: csA | csB | tri512 | ident | ones128
    CPK = 2 * S + 512 + 2 * DH
    cpk_d = nc.dram_tensor("cpk", [DH, CPK], BF16, kind="ExternalInput")
    sign_d = nc.dram_tensor("sign", [DH, 1], F32, kind="ExternalInput")
    out_d = nc.dram_tensor("out", [S, DIM], F32, kind="ExternalOutput")

    with tile.TileContext(nc) as tc:
        with (
            tc.tile_pool(name="consts", bufs=1) as cpool,
            tc.tile_pool(name="persist", bufs=1) as ppool,
            tc.tile_pool(name="xin", bufs=2) as xpool,
            tc.tile_pool(name="ropet", bufs=2) as rtp,
            tc.tile_pool(name="rawsb", bufs=5) as rawpool,
            tc.tile_pool(name="vtt", bufs=1) as vtp,
        ):
            cpk_sb = cpool.tile([DH, CPK], BF16)
            csA_sb = cpk_sb[:, 0:S]
            csB_sb = cpk_sb[:, S:2 * S]
            tri512_sb = cpk_sb[:, 2 * S:2 * S + 512]
            ident_sb = cpk_sb[:, 2 * S + 512:2 * S + 512 + DH]
            ones128_sb = cpk_sb[:, 2 * S + 512 + DH:2 * S + 512 + 2 * DH]
            sign_sb = cpool.tile([DH, 1], F32)

            qt = [ppool.tile([DH, S], BF16, name=f"qt{h}") for h in range(HPC)]
            kt = ppool.tile([DH, S], BF16)
            vn = ppool.tile([DH, S], BF16)  # V normal layout, 16 [128,128] chunks
            wo_sb = ppool.tile([DH, HPC * DIM], BF16)
            # otn aliases qt: attention block b is the last reader of
            # qt[h][:, b*512:(b+1)*512], so the normalized out^T overwrites it.
            otn = qt

            def emit_rope(f, sb_i, raw):
                # head_dim permuted so pairs sit +-16 apart within each
                # 32-partition quadrant: dest = p1 + sign*p3 where
                # p1 = q*cos, p3 = halfswap(q)*sin. All bf16 SBUF operands so
                # the muls run 2x and the stt 4x on the DVE.
                col = slice(sb_i * 512, (sb_i + 1) * 512)
                dest = qt[f] if f < HPC else kt
                qs_t = rtp.tile([DH, 512], BF16, name="qs_t", tag="qs")
                p1 = rtp.tile([DH, 512], BF16, name="p1", tag="p1")
                nc.vector.stream_shuffle(qs_t[:], raw[:], _SWAP16)
                nc.vector.tensor_mul(p1[:], raw[:], csA_sb[:, col])
                nc.vector.tensor_mul(qs_t[:], qs_t[:], csB_sb[:, col])
                nc.vector.scalar_tensor_tensor(
                    dest[:, col], qs_t[:], sign_sb[:], p1[:],
                    mybir.AluOpType.mult, mybir.AluOpType.add)

            # ---------------- Phase 1: QKV projections --------------------
            with (
                tc.tile_pool(name="mps", bufs=6, space="PSUM") as mps,
                tc.tile_pool(name="trps", bufs=1, space="PSUM") as trps,
                tc.tile_pool(name="wsb", bufs=1) as wpool,
            ):
                w_sb = wpool.tile([DH, NDCH * FQKV], BF16)

                def emit_sblock(sb_i):
                    # x consumed in multi-chunk groups: one DMA per group
                    # (contiguous in the host-shuffled x2 layout), alternating
                    # the SP-HWDGE and SWDGE rings. s-block 0 leads with a
                    # small group so the first matmul starts ASAP, and its
                    # w groups stream on the ACT ring in matching sizes.
                    groups = ([(0, 2), (2, 6), (6, 14), (14, 22), (22, 32)]
                              if sb_i == 0 else
                              [(0, 8), (8, 16), (16, 24), (24, 32)])
                    ps = [mps.tile([DH, 512], F32, name=f"ps{f}", tag="ps")
                          for f in range(6)]
                    for gi, (a, b) in enumerate(groups):
                        xg = xpool.tile([DH, (b - a) * 512], BF16, name="xg",
                                        tag="xg")
                        xeng = nc.sync if (sb_i + gi) % 2 == 0 else nc.gpsimd
                        xo = sb_i * NDCH * 512
                        xeng.dma_start(xg[:],
                                       x2_d[:, xo + a * 512:xo + b * 512])
                        if sb_i == 0:
                            nc.scalar.dma_start(
                                w_sb[:, a * FQKV:b * FQKV],
                                wq2_d[:, a * FQKV:b * FQKV])
                            if gi == len(groups) - 1:
                                nc.scalar.dma_start(cpk_sb[:], cpk_d[:])
                                nc.scalar.dma_start(sign_sb[:], sign_d[:])
                        if sb_i == 1 and gi < 2:
                            # wo (4MB bf16) in two halves on the ACT ring,
                            # after s-block 0's bus traffic has drained
                            half = HPC * DIM // 2
                            nc.scalar.dma_start(
                                wo_sb[:, gi * half:(gi + 1) * half],
                                wo2_d[:, gi * half:(gi + 1) * half])
                        for d in range(a, b):
                            xcol = slice((d - a) * 512, (d - a + 1) * 512)
                            for f in range(6):
                                nc.tensor.matmul(
                                    ps[f][:],
                                    w_sb[:, d * FQKV + f * DH:
                                         d * FQKV + (f + 1) * DH],
                                    xg[:, xcol], start=(d == 0),
                                    stop=(d == NDCH - 1))
                    # Fast raw PSUM->SBUF evictions (alternating ACT/DVE) free
                    # the accumulator banks quickly; RoPE runs later from SBUF.
                    vt_t = vtp.tile([DH, 512], BF16, name="vt_t", tag="vt")
                    nc.scalar.copy(vt_t[:], ps[5][:])
                    raws = {}
                    for i, f in enumerate([0, 4, 2, 1, 3]):
                        raw = rawpool.tile([DH, 512], BF16, name="raw", tag="raw")
                        raws[f] = raw
                        if i % 2 == 1:
                            nc.scalar.copy(raw[:], ps[f][:])
                        else:
                            nc.vector.tensor_copy(raw[:], ps[f][:])
                    for t in range(4):
                        tp = trps.tile([DH, DH], BF16, name="tp", tag="tp")
                        nc.tensor.transpose(tp[:], vt_t[:, t * DH:(t + 1) * DH],
                                            ident_sb[:])
                        j = sb_i * 4 + t
                        nc.vector.tensor_copy(vn[:, j * DH:(j + 1) * DH], tp[:])
                    return raws

                for sb_i in range(NQB):
                    raws = emit_sblock(sb_i)
                    if sb_i < NQB - 1:
                        for f in [0, 4, 1, 2, 3]:
                            emit_rope(f, sb_i, raws[f])

            # ---------------- Phase 2: attention + out-proj pipelined ------
            with (
                tc.tile_pool(name="spps", bufs=3, space="PSUM") as spps,
                tc.tile_pool(name="otps", bufs=2, space="PSUM") as otps,
                tc.tile_pool(name="pwps", bufs=1, space="PSUM") as pwps,
                tc.tile_pool(name="esb", bufs=4) as epool,
                tc.tile_pool(name="essb", bufs=2) as espool,
                tc.tile_pool(name="bcsb", bufs=2) as bcpool,
                tc.tile_pool(name="evsb", bufs=3) as evpool,
            ):
                class OpjEmitter:
                    """Output projection for s-tiles of q-block bprev, emitted
                    one matmul per step() so the attention emitter can pace it.
                    Unit = (st, dh_i, jj): 8 matmuls (4 heads x 2 adjacent
                    512-wide dout cols, stationary otn[h] shared), 2 PSUM
                    evictions (ACT/DVE), one [128,1024] store. Plain state
                    machine (not a generator): tile-pool allocs from a
                    suspended generator frame break the pool's scope-matched
                    reuse dependencies."""

                    def __init__(self, bprev, u0):
                        self.units = [(st, dh_i, jj)
                                      for st in range(4 * bprev, 4 * bprev + 4)
                                      for dh_i in range(2) for jj in range(2)]
                        self.ui = 0
                        self.mi = 0
                        self.u = u0
                        self.pw0 = self.pw1 = None

                    def step(self):
                        if self.ui >= len(self.units):
                            return False
                        st, dh_i, jj = self.units[self.ui]
                        scol = slice(st * DH, (st + 1) * DH)
                        base = dh_i * 2048 + jj * 1024
                        if self.mi == 0:
                            self.pw0 = pwps.tile([DH, 512], F32, name="pw0",
                                                 tag="pw0", bufs=2)
                            self.pw1 = pwps.tile([DH, 512], F32, name="pw1",
                                                 tag="pw1", bufs=1)
                        # j-major: pw0's accumulation (4 matmuls) completes
                        # first and evicts while pw1's matmuls run; pw1's
                        # next-unit reuse then trails its eviction by 4
                        # matmuls, so a single pw1 buffer never stalls the PE
                        j2, h2 = divmod(self.mi, HPC)
                        o = h2 * DIM + base + j2 * 512
                        pw = self.pw0 if j2 == 0 else self.pw1
                        nc.tensor.matmul(
                            pw[:], otn[h2][:, scol], wo_sb[:, o:o + 512],
                            start=(h2 == 0), stop=(h2 == HPC - 1))
                        self.mi += 1
                        if self.mi == HPC:
                            self.ev = evpool.tile([DH, 1024], F32, name="ev",
                                                  tag="ev")
                            if self.u % 2 == 0:
                                nc.scalar.copy(self.ev[:, 0:512], self.pw0[:])
                            else:
                                nc.vector.tensor_copy(self.ev[:, 0:512],
                                                      self.pw0[:])
                        if self.mi == 8:
                            ev = self.ev
                            if self.u % 2 == 0:
                                nc.vector.tensor_copy(ev[:, 512:1024],
                                                      self.pw1[:])
                            else:
                                nc.scalar.copy(ev[:, 512:1024], self.pw1[:])
                            dst = out_d[scol, base:base + 1024]
                            if self.ui >= len(self.units) - 4:
                                # split the final stores across both HWDGE
                                # rings to shrink the kernel tail
                                nc.sync.dma_start(
                                    out_d[scol, base:base + 512], ev[:, 0:512])
                                nc.scalar.dma_start(
                                    out_d[scol, base + 512:base + 1024],
                                    ev[:, 512:1024])
                            else:
                                # stores alternate the two HWDGE rings; the
                                # SWDGE (gpsimd) ring is too slow for stores
                                (nc.sync if self.u % 2 == 0 else nc.scalar
                                 ).dma_start(dst, ev[:])
                            self.u += 1
                            self.mi = 0
                            self.ui += 1
                        return True

                prev_tail = [None]

                def emit_block(h, b, fill):
                    cb = slice(b * 512, (b + 1) * 512)
                    nk = 4 * b + 4  # k tiles contributing to this q block
                    ot_b = otps.tile([DH, 512], F32, name="ot", tag="ot")
                    esum = espool.tile([DH, 512], BF16, name="esum", tag="es")
                    e_tiles = [None] * nk

                    def emit_scores(k):
                        e = epool.tile([DH, 512], BF16, name="E", tag="E")
                        e_tiles[k] = e
                        sp = spps.tile([DH, 512], F32, name="sp", tag="sp")
                        nc.tensor.matmul(
                            sp[:], kt[:, k * DH:(k + 1) * DH],
                            qt[h][:, cb], start=True, stop=True)
                        nc.scalar.activation(e[:], sp[:], EXP)
                        if k // 4 == b:
                            # diagonal block: zero the disallowed prefix
                            w = (k % 4 + 1) * DH
                            nc.vector.tensor_mul(
                                e[:, :w], e[:, :w], tri512_sb[:, 512 - w:])
                        # accumulate the softmax denominator on the DVE
                        if k == 0:
                            nc.vector.tensor_copy(esum[:], e[:])
                        else:
                            nc.vector.tensor_add(esum[:], esum[:], e[:])

                    def emit_pv(k):
                        e = e_tiles[k]
                        st_, sp_ = (k == 0), (k == nk - 1)
                        nc.tensor.matmul(ot_b[:], vn[:, k * DH:(k + 1) * DH],
                                         e[:], start=st_, stop=sp_)

                    # 2-deep software pipeline: scores run two steps ahead of
                    # PV so exp/mask latency never stalls the PE; out-proj
                    # matmuls of block b-1 are interleaved to absorb ACT lag
                    emit_scores(0)
                    if nk > 1:
                        emit_scores(1)
                    # the previous block's normalization tail lands here: by
                    # now its exp->esum chain has long drained, so its
                    # ones-matmul never stalls the PE
                    if prev_tail[0] is not None:
                        prev_tail[0]()
                        prev_tail[0] = None
                    fill(1)
                    for k in range(2, nk):
                        emit_scores(k)
                        emit_pv(k - 2)
                        fill(1)
                    if nk > 1:
                        emit_pv(nk - 2)
                    emit_pv(nk - 1)
                    fill(4)

                    def tail():
                        # denominator: one ones-matmul reduces esum across
                        # partitions (every PSUM row = colsum -> broadcast
                        # for free), then fast reciprocal + fused normalize.
                        dn_b = spps.tile([DH, 512], F32, name="dn", tag="sp")
                        nc.tensor.matmul(dn_b[:], ones128_sb[:], esum[:],
                                         start=True, stop=True)
                        bc_sb = bcpool.tile([DH, 512], F32, name="bc_sb",
                                            tag="bcs")
                        nc.vector.reciprocal_approx_fast(out=bc_sb[:],
                                                         in_=dn_b[:])
                        nc.vector.tensor_mul(otn[h][:, cb], ot_b[:], bc_sb[:])

                    prev_tail[0] = tail

                gen = None

                def fill(n):
                    if gen is None:
                        return
                    for _ in range(n):
                        if not gen.step():
                            break

                def no_fill(n):
                    pass

                # last s-block's RoPE: kt and qt[0] first (needed by b<=3 /
                # b=3 of head 0), the rest spread between early blocks
                emit_rope(4, 3, raws[4])
                emit_rope(0, 3, raws[0])
                for b in range(NQB):
                    gen = OpjEmitter(b - 1, 16 * (b - 1)) if b >= 1 else None
                    for h in range(HPC):
                        if b == 0 and h >= 1:
                            emit_rope(h, 3, raws[h])
                        # h=0: block b-1's last head normalizes during these
                        # scores, so its otn isn't ready for out-proj yet
                        emit_block(h, b, no_fill if h == 0 else fill)
                    fill(1 << 30)  # drain the rest of block b-1's out-proj
                if prev_tail[0] is not None:
                    prev_tail[0]()
                    prev_tail[0] = None
                gen = OpjEmitter(NQB - 1, 16 * (NQB - 1))
                fill(1 << 30)  # PE-dense tail

    nc.compile()
    return nc


def get_program():
    global _PROGRAM
    if _PROGRAM is None:
        _PROGRAM = _build_program()
    return _PROGRAM


def make_in_maps(inputs):
    """Host-side sharding / layout prep. Returns one input dict per core."""
    import ml_dtypes
    bf16 = ml_dtypes.bfloat16

    x = np.asarray(inputs["x"], dtype=np.float32)
    wq = np.asarray(inputs["wq"], dtype=np.float32)
    wk = np.asarray(inputs["wk"], dtype=np.float32)
    wv = np.asarray(inputs["wv"], dtype=np.float32)
    wo = np.asarray(inputs["wo"], dtype=np.float32)
    cos = np.asarray(inputs["freqs_cos"], dtype=np.float32)  # (S, 64)
    sin = np.asarray(inputs["freqs_sin"], dtype=np.float32)

    xT = x.reshape(S, DIM).T  # (DIM, S)
    # x2: SBUF-mirror layout [128, (sb, d, col)] so each x group load is one
    # contiguous-row DMA: x2[p, sb*32*512 + d*512 + c] = xT[d*128+p, sb*512+c]
    x2 = np.ascontiguousarray(
        xT.reshape(NDCH, DH, NQB, 512).transpose(1, 2, 0, 3).reshape(
            DH, NQB * NDCH * 512)).astype(bf16)

    perm = _head_perm()
    sq = np.float32(DH ** -0.25)  # sqrt of 1/sqrt(head_dim), folded into Q and K
    rows = np.arange(DH)
    pair_idx = 16 * (rows // 32) + (rows % 32) % 16
    csA = np.ascontiguousarray(cos.T[pair_idx] * sq)   # (128, S)
    csB = np.ascontiguousarray(sin.T[pair_idx] * sq)
    sign = np.where((rows % 32) < 16, -1.0, 1.0).astype(np.float32).reshape(DH, 1)
    tri = np.triu(np.ones((DH, DH), dtype=np.float32))
    tri512 = np.concatenate([np.zeros((DH, 512 - DH), np.float32), tri], axis=1)
    ident = np.eye(DH, dtype=np.float32)
    ones128 = np.ones((DH, DH), dtype=np.float32)
    # consts pack mirrors the cpk_sb slices: csA | csB | tri512 | ident | ones
    cpk = np.concatenate([csA, csB, tri512, ident, ones128],
                         axis=1).astype(bf16)

    wqh = wq.reshape(N_HEADS, DH, DIM)[:, perm, :]
    wkh = wk.reshape(N_KV, DH, DIM)[:, perm, :]
    wvh = wv.reshape(N_KV, DH, DIM)

    in_maps = []
    for c in range(NCORES):
        w_c = np.concatenate(
            [wqh[HPC * c:HPC * (c + 1)].reshape(HPC * DH, DIM),
             wkh[c], wvh[c]], 0)  # (768, DIM)
        wqkvT = w_c.T  # (DIM, 768)
        # wq2: SBUF-mirror [128, (d, f)]: wq2[p, d*768+f] = wqkvT[d*128+p, f]
        wq2 = np.ascontiguousarray(
            wqkvT.reshape(NDCH, DH, FQKV).transpose(1, 0, 2).reshape(
                DH, NDCH * FQKV)).astype(bf16)
        woT = wo[:, HPC * DH * c:HPC * DH * (c + 1)].T  # (512, DIM)
        # wo2: SBUF-mirror [128, (ch, dout)]
        wo2 = np.ascontiguousarray(
            woT.reshape(HPC, DH, DIM).transpose(1, 0, 2).reshape(
                DH, HPC * DIM)).astype(bf16)
        in_maps.append({
            "x2": x2, "wq2": wq2, "wo2": wo2, "cpk": cpk, "sign": sign,
        })
    return in_maps


def _ensure_ntff_hook():
    """The agent image's `antenv` lacks `axon_hooks`; recreate it so
    run_bass_kernel_spmd(trace=True) can capture NTFF profiles."""
    import sys
    try:
        from antenv.axon_hooks import get_axon_ntff_profile_hook  # noqa: F401
        return
    except ImportError:
        pass
    import contextlib
    import ctypes
    import types

    so_path = "/opt/axon/libaxon_pjrt.so"
    hook = None
    try:
        lib = ctypes.CDLL(so_path)
        if hasattr(lib, "axon_start_nrt_profile"):
            lib.axon_start_nrt_profile.argtypes = [
                ctypes.POINTER(ctypes.c_int64), ctypes.c_size_t]
            lib.axon_start_nrt_profile.restype = ctypes.c_int64
            lib.axon_stop_nrt_profile.argtypes = [ctypes.c_char_p]
            lib.axon_stop_nrt_profile.restype = ctypes.c_int64

            @contextlib.contextmanager
            def _hook(output_dir, device_ids):
                import jax
                jax.devices()
                if device_ids:
                    ids = (ctypes.c_int64 * len(device_ids))(*device_ids)
                    rc = lib.axon_start_nrt_profile(ids, len(device_ids))
                else:
                    rc = lib.axon_start_nrt_profile(None, 0)
                if rc != 0:
                    raise RuntimeError(f"axon_start_nrt_profile rc={rc}")
                try:
                    yield
                finally:
                    n = lib.axon_stop_nrt_profile(str(output_dir).encode())
                    print(f"profile: {n} file(s) written to {output_dir}")

            hook = _hook
    except OSError:
        pass

    mod = types.ModuleType("antenv.axon_hooks")
    mod._hook = hook
    mod.get_axon_ntff_profile_hook = lambda: mod._hook
    mod.set_axon_ntff_profile_hook = lambda h: setattr(mod, "_hook", h)
    sys.modules["antenv.axon_hooks"] = mod


def run(inputs, trace=False):
    from concourse.bass_utils import run_bass_kernel_spmd
    if trace:
        _ensure_ntff_hook()
    nc = get_program()
    in_maps = make_in_maps(inputs)
    res = run_bass_kernel_spmd(nc, in_maps, core_ids=list(range(NCORES)),
                               trace=trace)
    acc = np.zeros((S, DIM), dtype=np.float32)
    for r in res.results:
        acc += np.asarray(r["out"], dtype=np.float32)
    return acc.reshape(B, S, DIM), res


def kernel(**inputs):
    out, _ = run(inputs, trace=False)
    return out
